# revision 7
# baseline (speedup 1.0000x reference)
"""Cross-attention Trainium2 kernel (8 NeuronCores, SPMD).

Problem: B=4, C=256, H=W=64 -> N=4096 tokens/batch, single-head attention
over full C=256 with scale 1/sqrt(64)=1/8, then output projection.

Sharding: 2 cores per batch; each core owns 2048 queries (half the batch's
4096) and replicates K/V work for its batch (cheap vs. collectives).

v2 layout strategy (all fp32r matmuls):
  - Wk folded into the Q projection on the host (softmax is invariant to
    the per-query cross term), Wo folded into Wv.  wq/wv/bq ship as one
    packed weight tensor W [256, 514].
  - V built in [key, d] layout with an appended ones-column so the AV
    matmul also produces the softmax denominator.
  - V projection chunks are fused into group 0's attention loop so the
    PSUM->SBUF copies (DVE/Pool) hide behind scores/AV matmuls.
  - Output ships unnormalized: numerator [q, 256] + denominator (col 256)
    straight to DRAM; the host does divide + bias + transpose.  No PE
    transposes, no device-side normalization.
"""

import numpy as np

B, C, HW = 4, 256, 4096
NQ = HW // 2          # queries per core
NCORES = 8
KC = HW // 128        # 32 key chunks
QG = NQ // 512        # 4 query groups of 512 per core
SCALE = 1.0 / 8.0     # 1/sqrt(dim_head=64)

_COMPILED = {}


def _build_nc():
    import concourse.bass as bass
    from concourse import bacc, mybir
    import concourse.tile as tile

    dt = mybir.dt.float32
    rdt = mybir.dt.float32r
    Exp = mybir.ActivationFunctionType.Exp

    nc = bacc.Bacc("TRN2", target_bir_lowering=False, debug=False)

    aT = nc.dram_tensor("aT", [C, NQ], rdt, kind="ExternalInput")
    bT = nc.dram_tensor("bT", [C, HW], rdt, kind="ExternalInput")
    wd = nc.dram_tensor("w", [C, 514], rdt, kind="ExternalInput")
    out = nc.dram_tensor("out", [NQ, 258], dt, kind="ExternalOutput")

    with tile.TileContext(nc) as tc:
        with (
            tc.tile_pool(name="consts", bufs=1) as consts,
            tc.tile_pool(name="feat", bufs=1) as feat,
            tc.tile_pool(name="qkt", bufs=1) as qkt,
            tc.tile_pool(name="vsb", bufs=1) as vsb,
            tc.tile_pool(name="expp", bufs=3) as expp,
            tc.tile_pool(name="ostg", bufs=1) as ostg,
            tc.tile_pool(name="vp_ps", bufs=2, space="PSUM") as vp_ps,
            tc.tile_pool(name="s_ps", bufs=2, space="PSUM") as s_ps,
            tc.tile_pool(name="o_ps", bufs=1, space="PSUM") as o_ps,
        ):
            w_sb = [consts.tile([128, 514], rdt, tag=f"w{j}", name=f"w{j}")
                    for j in range(2)]
            bt_sb = [feat.tile([128, HW], rdt, tag=f"bt{j}", name=f"bt{j}")
                     for j in range(2)]
            at_sb = [feat.tile([128, NQ], rdt, tag=f"at{j}", name=f"at{j}")
                     for j in range(2)]
            # qt per (do, group) as separate tiles for fine-grained deps
            qt_sb = [[qkt.tile([128, 512], rdt, tag=f"qt{j}g{g}",
                               name=f"qt{j}g{g}") for g in range(QG)]
                     for j in range(2)]
            v_sb = [vsb.tile([128, 258], rdt, tag=f"v{k}", name=f"v{k}")
                    for k in range(KC)]

            # ---- DMA program (SP queue, in issue order) ----
            def dma(dst, src):
                nc.sync.dma_start(out=dst, in_=src)

            dma(w_sb[0], wd[0:128, :])
            dma(w_sb[1], wd[128:256, :])
            for j in range(2):
                dma(bt_sb[j][:, 0:256], bT[j * 128:(j + 1) * 128, 0:256])
            for j in range(2):
                dma(at_sb[j][:, 0:512], aT[j * 128:(j + 1) * 128, 0:512])
            for j in range(2):
                dma(bt_sb[j][:, 256:1280], bT[j * 128:(j + 1) * 128, 256:1280])
            for j in range(2):
                dma(at_sb[j][:, 512:1280], aT[j * 128:(j + 1) * 128, 512:1280])
            for j in range(2):
                dma(bt_sb[j][:, 1280:2304], bT[j * 128:(j + 1) * 128, 1280:2304])
            for j in range(2):
                dma(at_sb[j][:, 1280:2048], aT[j * 128:(j + 1) * 128, 1280:2048])
            for j in range(2):
                dma(bt_sb[j][:, 2304:3328], bT[j * 128:(j + 1) * 128, 2304:3328])
            for j in range(2):
                dma(bt_sb[j][:, 3328:4096], bT[j * 128:(j + 1) * 128, 3328:4096])

            # ones columns for the AV denominator; ACT table warm-up
            ones = consts.tile([128, 2], dt, tag="ones")
            nc.vector.memset(ones, 1.0)
            for k in range(KC):
                nc.vector.tensor_copy(v_sb[k][:, 256:258], ones)
            warm = consts.tile([128, 1], dt, tag="warm")
            nc.scalar.activation(out=warm, in_=ones[:, 0:1], func=Exp)

            # ---- emission helpers ----
            vcnt = [0]

            def vproj(k):
                ps = vp_ps.tile([128, 256], dt, tag="vp", name=f"vp{k}")
                for di in range(2):
                    nc.tensor.matmul(
                        ps,
                        bt_sb[di][:, k * 128:(k + 1) * 128],
                        w_sb[di][:, 0:256],
                        start=(di == 0), stop=(di == 1),
                    )
                if vcnt[0] % 2 == 0:
                    nc.vector.tensor_copy(v_sb[k][:, 0:256], ps)
                else:
                    nc.scalar.copy(v_sb[k][:, 0:256], ps)
                vcnt[0] += 1

            def qproj(g):
                for do in range(2):
                    ps = s_ps.tile([128, 512], dt, tag="sp", name=f"qp{g}{do}")
                    for di in range(2):
                        nc.tensor.matmul(
                            ps,
                            w_sb[di][:, 256 + do * 128:256 + (do + 1) * 128],
                            at_sb[di][:, g * 512:(g + 1) * 512],
                            start=(di == 0), stop=(di == 1),
                        )
                    nc.vector.tensor_scalar_add(
                        qt_sb[do][g], ps, w_sb[do][:, 512:513].bitcast(dt))

            ets = [None] * KC

            def scores(g, k):
                sp = s_ps.tile([128, 512], dt, tag="sp", name=f"s{g}_{k}")
                for di in range(2):
                    nc.tensor.matmul(
                        sp,
                        bt_sb[di][:, k * 128:(k + 1) * 128],
                        qt_sb[di][g],
                        start=(di == 0), stop=(di == 1),
                    )
                et = expp.tile([128, 512], rdt, tag="et", name=f"e{g}_{k}")
                nc.scalar.activation(out=et, in_=sp, func=Exp)
                ets[k] = et

            o_acc = [o_ps.tile([128, 258], dt, tag=f"o{qs}", name=f"o{qs}")
                     for qs in range(4)]

            def av(g, k):
                for qs in range(4):
                    nc.tensor.matmul(
                        o_acc[qs],
                        ets[k][:, qs * 128:(qs + 1) * 128],
                        v_sb[k][:, 0:258],
                        start=(k == 0), stop=(k == KC - 1),
                    )
                ets[k] = None

            def flush(g):
                for qs in range(4):
                    ot = ostg.tile([128, 258], dt, tag=f"ot{qs}",
                                   name=f"ot{g}{qs}")
                    if qs % 2 == 0:
                        nc.vector.tensor_copy(ot, o_acc[qs])
                    else:
                        nc.scalar.copy(ot, o_acc[qs])
                    nc.sync.dma_start(
                        out=out[(g * 4 + qs) * 128:(g * 4 + qs + 1) * 128, :],
                        in_=ot)

            # ---- schedule ----
            vproj(0)
            vproj(1)
            qproj(0)
            vproj(2)
            vproj(3)
            for g in range(QG):
                for k in range(KC):
                    scores(g, k)
                    if k >= 1:
                        av(g, k - 1)
                    if g == 0 and k + 4 < KC:
                        vproj(k + 4)
                    if k == 20 and g < QG - 1:
                        qproj(g + 1)
                av(g, KC - 1)
                flush(g)
    nc.finalize()
    return nc


def _get_nc():
    if "nc" not in _COMPILED:
        _COMPILED["nc"] = _build_nc()
    return _COMPILED["nc"]


def _get_runner():
    """Jit the SPMD executable once and reuse it across kernel() calls."""
    if "runner" in _COMPILED:
        return _COMPILED["runner"]
    import jax
    from jax.experimental.shard_map import shard_map
    from jax.sharding import Mesh, PartitionSpec
    from concourse import bass2jax, mybir
    from concourse.bass2jax import _bass_exec_p, install_neuronx_cc_hook

    nc = _get_nc()
    install_neuronx_cc_hook()
    try:
        jax.config.update("jax_compilation_cache_dir", "/tmp/jax_cache")
        jax.config.update("jax_persistent_cache_min_compile_time_secs", 0.0)
        jax.config.update("jax_persistent_cache_min_entry_size_bytes", -1)
    except Exception:
        pass
    in_names, out_names, out_avals, zero_outs = [], [], [], []
    for alloc in nc.m.functions[0].allocations:
        if not isinstance(alloc, mybir.MemoryLocationSet):
            continue
        name = alloc.memorylocations[0].name
        if alloc.kind == "ExternalInput":
            if nc.partition_id_tensor is None or \
                    name != nc.partition_id_tensor.name:
                in_names.append(name)
        elif alloc.kind == "ExternalOutput":
            out_names.append(name)
            shape = tuple(alloc.tensor_shape)
            dtype = mybir.dt.np(alloc.dtype)
            out_avals.append(jax.core.ShapedArray(shape, dtype))
            zero_outs.append(np.zeros(shape, dtype))
    all_names = in_names + out_names
    if nc.partition_id_tensor is not None:
        all_names.append(nc.partition_id_tensor.name)

    def _body(*args):
        operands = list(args)
        if nc.partition_id_tensor is not None:
            operands.append(bass2jax.partition_id_tensor())
        return tuple(_bass_exec_p.bind(
            *operands, out_avals=tuple(out_avals), in_names=tuple(all_names),
            out_names=tuple(out_names), lowering_input_output_aliases=(),
            sim_require_finite=True, sim_require_nnan=True, nc=nc))

    devices = jax.devices()[:NCORES]
    mesh = Mesh(np.asarray(devices), ("core",))
    n_io = len(in_names) + len(out_names)
    sharded = jax.jit(
        shard_map(_body, mesh=mesh,
                  in_specs=(PartitionSpec("core"),) * n_io,
                  out_specs=(PartitionSpec("core"),) * len(out_names),
                  check_rep=False),
        keep_unused=True)
    _COMPILED["runner"] = (sharded, in_names, out_names, zero_outs)
    return _COMPILED["runner"]


def kernel(feat_A, feat_B, Wq, bq, Wk, bk, Wv, bv, Wo, bo, **_unused):
    f32 = np.float32
    fa = np.asarray(feat_A, f32).reshape(B, C, HW)
    fb = np.asarray(feat_B, f32).reshape(B, C, HW)
    # fold Wk into the Q projection and Wo into the V projection; the
    # (Q-bias . bk) cross term is a per-query constant, which softmax
    # ignores, so it is dropped exactly.  products in float64, rounded
    # once to fp32.
    Wq64 = np.asarray(Wq, np.float64) * SCALE
    Wk64 = np.asarray(Wk, np.float64)
    Wv64 = np.asarray(Wv, np.float64)
    Wo64 = np.asarray(Wo, np.float64)
    wq_t = (Wq64.T @ Wk64).astype(f32)              # [C(in), C(do)]
    wv_t = (Wo64 @ Wv64).T.astype(f32)              # [C(in), C(d)]
    bq_s = ((np.asarray(bq, np.float64) * SCALE) @ Wk64).astype(f32)
    bv_r = (Wo64 @ np.asarray(bv, np.float64)).astype(f32)   # [C]
    bo_c = np.asarray(bo, f32)                      # [C]

    wpack = np.zeros((C, 514), f32)
    wpack[:, 0:256] = wv_t
    wpack[:, 256:512] = wq_t
    wpack[:, 512] = bq_s

    in_maps = []
    for cidx in range(NCORES):
        b, qh = cidx // 2, cidx % 2
        in_maps.append({
            "aT": np.ascontiguousarray(fa[b][:, qh * NQ:(qh + 1) * NQ]),
            "bT": np.ascontiguousarray(fb[b]),
            "w": wpack,
        })

    try:
        sharded, in_names, out_names, zero_outs = _get_runner()
        concat_in = [np.concatenate([in_maps[c][nm] for c in range(NCORES)],
                                    axis=0) for nm in in_names]
        concat_zeros = [np.zeros((NCORES * z.shape[0], *z.shape[1:]), z.dtype)
                        for z in zero_outs]
        out_arrs = sharded(*concat_in, *concat_zeros)
        res_out = np.asarray(out_arrs[out_names.index("out")]) \
            .reshape(NCORES, NQ, 258)
    except Exception:
        from concourse.bass_utils import run_bass_kernel_spmd
        res = run_bass_kernel_spmd(_get_nc(), in_maps, list(range(NCORES)))
        res_out = np.stack([res.results[c]["out"] for c in range(NCORES)])

    add_c = (bv_r + bo_c).astype(f32)               # [C]
    outf = np.empty((B, C, HW), f32)
    for cidx in range(NCORES):
        b, qh = cidx // 2, cidx % 2
        num = res_out[cidx][:, 0:256]
        den = res_out[cidx][:, 256:257]
        tok = num / den + add_c                     # [NQ, C]
        outf[b][:, qh * NQ:(qh + 1) * NQ] = tok.T
    return outf.reshape(B, C, 64, 64)


if __name__ == "__main__":
    rng = np.random.default_rng(0)
    ins = {
        "feat_A": rng.standard_normal((B, C, 64, 64), dtype=np.float32),
        "feat_B": rng.standard_normal((B, C, 64, 64), dtype=np.float32),
    }
    for nm in ("q", "k", "v", "o"):
        ins[f"W{nm}"] = rng.standard_normal((C, C), dtype=np.float32) / 16.0
        ins[f"b{nm}"] = np.zeros(C, np.float32)
    o = kernel(**ins)
    print("kernel ran, out shape", o.shape, "mean", float(np.abs(o).mean()))


# revision 9
# speedup vs baseline: 1.0802x; 1.0802x over previous
"""Cross-attention Trainium2 kernel (8 NeuronCores, SPMD).

Problem: B=4, C=256, H=W=64 -> N=4096 tokens/batch, single-head attention
over full C=256 with scale 1/sqrt(64)=1/8, then output projection.

Sharding: 2 cores per batch; each core owns 2048 queries (half the batch's
4096) and replicates K/V work for its batch (cheap vs. collectives).

v4 strategy: split-precision fp8 DoubleRow matmuls for scores and both
projections; fp32r for AV.

  - Wk folded into Q projection on host, Wo folded into Wv.
  - Every fp8 operand x ships as three e4m3 tensors: hi=fp8(x),
    lo16=fp8((x-hi)*16), hi16=fp8(hi/16).  A product x@y is computed as
    3 DoubleRow matmuls: hi@hi + hi16@lo16 + lo16@hi16 (exact power-of-2
    scale cancellation; dropped lo*lo term is ~1e-4 relative).
  - DoubleRow contracts 2x128=256 in ONE instruction at 0.5 cyc/row, so
    each 3-term product costs 75% of the fp32r pair while keeping
    ~4e-3 end-to-end relative error (measured in sim_fp8.py).
  - Scale balance: scores operands q,k at sigma~0.354 (sqrt(1/8) each),
    qproj operands a*0.177 / M*2, vproj k_dev / wv_t/0.354.
  - AV stays fp32r with the ones-column denominator trick; numerator +
    denominator ship unnormalized to DRAM; host does divide + bias +
    transpose.
  - V projection in chunk pairs fused into group 0's attention loop.
"""

import numpy as np

B, C, HW = 4, 256, 4096
NQ = HW // 2          # queries per core
NCORES = 8
KC = HW // 128        # 32 key chunks
QG = NQ // 512        # 4 query groups of 512 per core
SCALE = 1.0 / 8.0     # 1/sqrt(dim_head=64)
SQ = float(np.sqrt(SCALE))

_COMPILED = {}


def _build_nc():
    import concourse.bass as bass
    from concourse import bacc, mybir
    import concourse.tile as tile

    dt = mybir.dt.float32
    rdt = mybir.dt.float32r
    e4 = mybir.dt.float8e4
    DR = mybir.MatmulPerfMode.DoubleRow
    Exp = mybir.ActivationFunctionType.Exp

    nc = bacc.Bacc("TRN2", target_bir_lowering=False, debug=False)

    # k/a packs: [128, 2, 3*N]: dim1 = channel half (ch = i*128+p),
    # dim2 = variant-major: [0:N]=hi, [N:2N]=lo16, [2N:3N]=hi16
    kp = nc.dram_tensor("kp", [128, 2, 3 * HW], e4, kind="ExternalInput")
    ap_d = nc.dram_tensor("ap", [128, 2, 3 * NQ], e4, kind="ExternalInput")
    # weight packs: [0:256]=wvh, [256:512]=wvl16, [512:768]=wvh16,
    #               [768:1024]=wqh, [1024:1280]=wql16, [1280:1536]=wqh16
    wp = nc.dram_tensor("wp", [128, 2, 1536], e4, kind="ExternalInput")
    bqd = nc.dram_tensor("bq", [C, 1], dt, kind="ExternalInput")
    out = nc.dram_tensor("out", [NQ, 258], dt, kind="ExternalOutput")

    with tile.TileContext(nc) as tc:
        with (
            tc.tile_pool(name="consts", bufs=1) as consts,
            tc.tile_pool(name="feat", bufs=1) as feat,
            tc.tile_pool(name="qkt", bufs=1) as qkt,
            tc.tile_pool(name="qpp", bufs=2) as qpp,
            tc.tile_pool(name="dqp", bufs=2) as dqp,
            tc.tile_pool(name="vsb", bufs=1) as vsb,
            tc.tile_pool(name="expp", bufs=4) as expp,
            tc.tile_pool(name="ostg", bufs=1) as ostg,
            tc.tile_pool(name="vp_ps", bufs=1, space="PSUM") as vp_ps,
            tc.tile_pool(name="s_ps", bufs=3, space="PSUM") as s_ps,
            tc.tile_pool(name="o_ps", bufs=1, space="PSUM") as o_ps,
        ):
            kp_sb = feat.tile([128, 2, 3 * HW], e4, tag="kp", name="kp")
            ap_sb = feat.tile([128, 2, 3 * NQ], e4, tag="ap", name="ap")
            wp_sb = consts.tile([128, 2, 1536], e4, tag="wp", name="wp")
            bq_sb = [consts.tile([128, 1], dt, tag=f"bq{j}", name=f"bq{j}")
                     for j in range(2)]
            qt_sb = [[qkt.tile([128, 512], dt, tag=f"qt{j}g{g}",
                               name=f"qt{j}g{g}") for g in range(QG)]
                     for j in range(2)]
            v_sb = [vsb.tile([128, 258], rdt, tag=f"v{k}", name=f"v{k}")
                    for k in range(KC)]

            def kvar(v, c0, c1):
                return kp_sb[:, :, v * HW + c0:v * HW + c1]

            def avar(v, g):
                return ap_sb[:, :, v * NQ + g * 512:v * NQ + (g + 1) * 512]

            def wvar(idx):
                return wp_sb[:, :, idx * 256:(idx + 1) * 256]

            # ---- DMA program ----
            # gpsimd(SWDGE) queue: weights + a-pack g0 + bq (parallel with
            # the SP/HWDGE k-pack stream)
            nc.gpsimd.dma_start(out=wp_sb, in_=wp[:, :, :])
            for v in range(3):
                nc.gpsimd.dma_start(
                    out=ap_sb[:, :, v * NQ:v * NQ + 512],
                    in_=ap_d[:, :, v * NQ:v * NQ + 512])
            for j in range(2):
                nc.gpsimd.dma_start(out=bq_sb[j],
                                    in_=bqd[j * 128:(j + 1) * 128, :])

            # SP queue: k-packs chunked, then a-pack tails
            def dma_k(c0, c1):
                for v in range(3):
                    nc.sync.dma_start(out=kvar(v, c0, c1),
                                      in_=kp[:, :, v * HW + c0:v * HW + c1])

            dma_k(0, 512)
            dma_k(512, 1536)
            for v in range(3):
                nc.sync.dma_start(
                    out=ap_sb[:, :, v * NQ + 512:(v + 1) * NQ],
                    in_=ap_d[:, :, v * NQ + 512:(v + 1) * NQ])
            dma_k(1536, 2560)
            dma_k(2560, 4096)

            # ones columns for the AV denominator; ACT table warm-up
            ones = consts.tile([128, 2], dt, tag="ones")
            nc.vector.memset(ones, 1.0)
            for k in range(KC):
                nc.vector.tensor_copy(v_sb[k][:, 256:258], ones)
            warm = consts.tile([128, 1], dt, tag="warm")
            nc.scalar.activation(out=warm, in_=ones[:, 0:1], func=Exp)

            # ---- emission helpers ----
            vcnt = [0]

            def vproj_pair(k):
                # chunks k, k+1 into one [128,512] PSUM tile; one copy out
                ps = vp_ps.tile([128, 512], dt, tag="vp", name=f"vp{k}")
                for kk in (k, k + 1):
                    sl = ps[:, (kk - k) * 256:(kk - k + 1) * 256]
                    nc.tensor.matmul(sl, kvar(0, kk * 128, (kk + 1) * 128),
                                     wvar(0), start=True, stop=False,
                                     perf_mode=DR)
                    nc.tensor.matmul(sl, kvar(2, kk * 128, (kk + 1) * 128),
                                     wvar(1), start=False, stop=False,
                                     perf_mode=DR)
                    nc.tensor.matmul(sl, kvar(1, kk * 128, (kk + 1) * 128),
                                     wvar(2), start=False, stop=True,
                                     perf_mode=DR)
                eng = nc.vector if vcnt[0] % 2 == 0 else nc.scalar
                vcnt[0] += 1
                if eng is nc.vector:
                    nc.vector.tensor_copy(v_sb[k][:, 0:256], ps[:, 0:256])
                    nc.vector.tensor_copy(v_sb[k + 1][:, 0:256], ps[:, 256:512])
                else:
                    nc.scalar.copy(v_sb[k][:, 0:256], ps[:, 0:256])
                    nc.scalar.copy(v_sb[k + 1][:, 0:256], ps[:, 256:512])

            def qproj(g):
                # wq packs at wvar indices 3,4,5; lhsT free dim = do chunk
                for do in range(2):
                    ps = s_ps.tile([128, 512], dt, tag="sp", name=f"qp{g}{do}")
                    d0, d1 = do * 128, (do + 1) * 128
                    nc.tensor.matmul(ps, wvar(3)[:, :, d0:d1], avar(0, g),
                                     start=True, stop=False, perf_mode=DR)
                    nc.tensor.matmul(ps, wvar(5)[:, :, d0:d1], avar(1, g),
                                     start=False, stop=False, perf_mode=DR)
                    nc.tensor.matmul(ps, wvar(4)[:, :, d0:d1], avar(2, g),
                                     start=False, stop=True, perf_mode=DR)
                    nc.vector.tensor_scalar_add(qt_sb[do][g], ps, bq_sb[do])

            qp = {}

            def qprep(g):
                # quantize qt -> qph/qpl16/qph16 [128, 2, 512] e4m3
                qph = qpp.tile([128, 2, 512], e4, tag="qph", name=f"qph{g}")
                qpl = qpp.tile([128, 2, 512], e4, tag="qpl", name=f"qpl{g}")
                qpu = qpp.tile([128, 2, 512], e4, tag="qpu", name=f"qpu{g}")
                for i in range(2):
                    hi = qph[:, i:i + 1, :]
                    nc.scalar.copy(hi, qt_sb[i][g])
                    dq = dqp.tile([128, 512], dt, tag="dq", name=f"dq{g}{i}")
                    nc.vector.tensor_sub(dq, qt_sb[i][g], hi)
                    nc.vector.tensor_scalar_mul(qpl[:, i:i + 1, :], dq, 16.0)
                    nc.scalar.mul(qpu[:, i:i + 1, :], hi, 0.0625)
                qp[g] = (qph, qpl, qpu)

            ets = [None] * KC

            def scores(g, k):
                qph, qpl, qpu = qp[g]
                sp = s_ps.tile([128, 512], dt, tag="sp", name=f"s{g}_{k}")
                c0, c1 = k * 128, (k + 1) * 128
                nc.tensor.matmul(sp, kvar(0, c0, c1), qph,
                                 start=True, stop=False, perf_mode=DR)
                nc.tensor.matmul(sp, kvar(2, c0, c1), qpl,
                                 start=False, stop=False, perf_mode=DR)
                nc.tensor.matmul(sp, kvar(1, c0, c1), qpu,
                                 start=False, stop=True, perf_mode=DR)
                et = expp.tile([128, 512], rdt, tag="et", name=f"e{g}_{k}")
                nc.scalar.activation(out=et, in_=sp, func=Exp)
                ets[k] = et

            o_acc = [o_ps.tile([128, 258], dt, tag=f"o{qs}", name=f"o{qs}")
                     for qs in range(4)]

            def av(g, k):
                for qs in range(4):
                    nc.tensor.matmul(
                        o_acc[qs],
                        ets[k][:, qs * 128:(qs + 1) * 128],
                        v_sb[k][:, 0:258],
                        start=(k == 0), stop=(k == KC - 1),
                    )
                ets[k] = None

            def flush(g):
                for qs in range(4):
                    ot = ostg.tile([128, 258], dt, tag=f"ot{qs}",
                                   name=f"ot{g}{qs}")
                    if qs % 2 == 0:
                        nc.vector.tensor_copy(ot, o_acc[qs])
                    else:
                        nc.scalar.copy(ot, o_acc[qs])
                    # split store issue between HWDGE (SP) and SWDGE (Pool)
                    eng = nc.sync if qs % 2 == 0 else nc.gpsimd
                    eng.dma_start(
                        out=out[(g * 4 + qs) * 128:(g * 4 + qs + 1) * 128, :],
                        in_=ot)

            # ---- schedule ----
            for k in range(0, 4, 2):
                vproj_pair(k)
            qproj(0)
            qprep(0)
            for k in range(4, 20, 2):
                vproj_pair(k)
            # group 0 (fused remaining vproj pairs), 2-deep scores pipeline
            next_v = [20]

            def maybe_vproj():
                if next_v[0] < KC:
                    vproj_pair(next_v[0])
                    next_v[0] += 2

            for g in range(QG):
                scores(g, 0)
                scores(g, 1)
                for k in range(KC):
                    if k >= 1:
                        av(g, k - 1)
                    if k + 2 < KC:
                        scores(g, k + 2)
                    if g == 0 and k % 4 == 1:
                        maybe_vproj()
                    if k == 16 and g < QG - 1:
                        qproj(g + 1)
                        qprep(g + 1)
                av(g, KC - 1)
                flush(g)
    nc.finalize()
    return nc


def _get_nc():
    if "nc" not in _COMPILED:
        _COMPILED["nc"] = _build_nc()
    return _COMPILED["nc"]


def _get_runner():
    """Jit the SPMD executable once and reuse it across kernel() calls."""
    if "runner" in _COMPILED:
        return _COMPILED["runner"]
    import jax
    from jax.experimental.shard_map import shard_map
    from jax.sharding import Mesh, PartitionSpec
    from concourse import bass2jax, mybir
    from concourse.bass2jax import _bass_exec_p, install_neuronx_cc_hook

    nc = _get_nc()
    install_neuronx_cc_hook()
    try:
        jax.config.update("jax_compilation_cache_dir", "/tmp/jax_cache")
        jax.config.update("jax_persistent_cache_min_compile_time_secs", 0.0)
        jax.config.update("jax_persistent_cache_min_entry_size_bytes", -1)
    except Exception:
        pass
    in_names, out_names, out_avals, zero_outs = [], [], [], []
    for alloc in nc.m.functions[0].allocations:
        if not isinstance(alloc, mybir.MemoryLocationSet):
            continue
        name = alloc.memorylocations[0].name
        if alloc.kind == "ExternalInput":
            if nc.partition_id_tensor is None or \
                    name != nc.partition_id_tensor.name:
                in_names.append(name)
        elif alloc.kind == "ExternalOutput":
            out_names.append(name)
            shape = tuple(alloc.tensor_shape)
            dtype = mybir.dt.np(alloc.dtype)
            out_avals.append(jax.core.ShapedArray(shape, dtype))
            zero_outs.append(np.zeros(shape, dtype))
    all_names = in_names + out_names
    if nc.partition_id_tensor is not None:
        all_names.append(nc.partition_id_tensor.name)

    def _body(*args):
        operands = list(args)
        if nc.partition_id_tensor is not None:
            operands.append(bass2jax.partition_id_tensor())
        return tuple(_bass_exec_p.bind(
            *operands, out_avals=tuple(out_avals), in_names=tuple(all_names),
            out_names=tuple(out_names), lowering_input_output_aliases=(),
            sim_require_finite=True, sim_require_nnan=True, nc=nc))

    devices = jax.devices()[:NCORES]
    mesh = Mesh(np.asarray(devices), ("core",))
    n_io = len(in_names) + len(out_names)
    sharded = jax.jit(
        shard_map(_body, mesh=mesh,
                  in_specs=(PartitionSpec("core"),) * n_io,
                  out_specs=(PartitionSpec("core"),) * len(out_names),
                  check_rep=False),
        keep_unused=True)
    _COMPILED["runner"] = (sharded, in_names, out_names, zero_outs)
    return _COMPILED["runner"]


def _split_pack(x):
    """x [128, 2, N] f32 -> concat([hi, lo16, hi16], axis=2) in e4m3."""
    import ml_dtypes
    E4 = ml_dtypes.float8_e4m3
    hi = x.astype(E4)
    hif = hi.astype(np.float32)
    lo16 = ((x - hif) * np.float32(16.0)).astype(E4)
    hi16 = (hif * np.float32(0.0625)).astype(E4)
    return np.concatenate([hi, lo16, hi16], axis=2)


def _chpack(x):
    """[C, N] -> [128, 2, N]: channel ch=i*128+p -> (p, i)."""
    return np.ascontiguousarray(
        x.reshape(2, 128, x.shape[1]).transpose(1, 0, 2))


def kernel(feat_A, feat_B, Wq, bq, Wk, bk, Wv, bv, Wo, bo, **_unused):
    f32 = np.float32
    fa = np.asarray(feat_A, f32).reshape(B, C, HW)
    fb = np.asarray(feat_B, f32).reshape(B, C, HW)
    # fold Wk into the Q projection and Wo into the V projection; the
    # (Q-bias . bk) cross term is a per-query constant, which softmax
    # ignores, so it is dropped exactly.
    Wq64 = np.asarray(Wq, np.float64) * SCALE
    Wk64 = np.asarray(Wk, np.float64)
    Wv64 = np.asarray(Wv, np.float64)
    Wo64 = np.asarray(Wo, np.float64)
    M = (Wq64.T @ Wk64) / SCALE                     # Wq^T Wk (unscaled)
    wv_t = (Wo64 @ Wv64).T                          # [C(in), C(d)]
    bq_s = (np.asarray(bq, np.float64) @ Wk64)
    bv_r = (Wo64 @ np.asarray(bv, np.float64)).astype(f32)
    bo_c = np.asarray(bo, f32)

    # device scaling: q_dev = (a*SA) @ (M*2) + bq*SQ  (sigma ~0.354)
    #                 k_dev = b*SQ;  v = k_dev^T @ (wv_t/SQ)
    SA = f32(SQ / 2)
    wq_dev = (M * 2.0).astype(f32)
    bq_dev = (bq_s * SQ).astype(f32).reshape(C, 1)
    wv_dev = (wv_t / SQ).astype(f32)

    wpack = np.concatenate([
        _split_pack(_chpack(wv_dev)),
        _split_pack(_chpack(wq_dev)),
    ], axis=2)

    # k packs are shared by the two cores of each batch
    kpacks = [_split_pack(_chpack((fb[b] * f32(SQ)))) for b in range(B)]

    in_maps = []
    for cidx in range(NCORES):
        b, qh = cidx // 2, cidx % 2
        a_half = fa[b][:, qh * NQ:(qh + 1) * NQ] * SA
        in_maps.append({
            "kp": kpacks[b],
            "ap": _split_pack(_chpack(a_half)),
            "wp": wpack,
            "bq": bq_dev,
        })

    try:
        sharded, in_names, out_names, zero_outs = _get_runner()
        concat_in = [np.concatenate([in_maps[c][nm] for c in range(NCORES)],
                                    axis=0) for nm in in_names]
        concat_zeros = [np.zeros((NCORES * z.shape[0], *z.shape[1:]), z.dtype)
                        for z in zero_outs]
        out_arrs = sharded(*concat_in, *concat_zeros)
        res_out = np.asarray(out_arrs[out_names.index("out")]) \
            .reshape(NCORES, NQ, 258)
    except Exception:
        from concourse.bass_utils import run_bass_kernel_spmd
        res = run_bass_kernel_spmd(_get_nc(), in_maps, list(range(NCORES)))
        res_out = np.stack([res.results[c]["out"] for c in range(NCORES)])

    add_c = (bv_r + bo_c).astype(f32)               # [C]
    outf = np.empty((B, C, HW), f32)
    for cidx in range(NCORES):
        b, qh = cidx // 2, cidx % 2
        num = res_out[cidx][:, 0:256]
        den = res_out[cidx][:, 256:257]
        tok = num / den + add_c                     # [NQ, C]
        outf[b][:, qh * NQ:(qh + 1) * NQ] = tok.T
    return outf.reshape(B, C, 64, 64)


if __name__ == "__main__":
    rng = np.random.default_rng(0)
    ins = {
        "feat_A": rng.standard_normal((B, C, 64, 64), dtype=np.float32),
        "feat_B": rng.standard_normal((B, C, 64, 64), dtype=np.float32),
    }
    for nm in ("q", "k", "v", "o"):
        ins[f"W{nm}"] = rng.standard_normal((C, C), dtype=np.float32) / 16.0
        ins[f"b{nm}"] = np.zeros(C, np.float32)
    o = kernel(**ins)
    print("kernel ran, out shape", o.shape, "mean", float(np.abs(o).mean()))


# revision 11
# speedup vs baseline: 1.1532x; 1.0675x over previous
"""Cross-attention Trainium2 kernel (8 NeuronCores, SPMD).

Problem: B=4, C=256, H=W=64 -> N=4096 tokens/batch, single-head attention
over full C=256 with scale 1/sqrt(64)=1/8, then output projection.

Sharding: 2 cores per batch; each core owns 2048 queries (half the batch's
4096) and replicates K/V work for its batch (cheap vs. collectives).

v4 strategy: split-precision fp8 DoubleRow matmuls for scores and both
projections; fp32r for AV.

  - Wk folded into Q projection on host, Wo folded into Wv.
  - Every fp8 operand x ships as three e4m3 tensors: hi=fp8(x),
    lo16=fp8((x-hi)*16), hi16=fp8(hi/16).  A product x@y is computed as
    3 DoubleRow matmuls: hi@hi + hi16@lo16 + lo16@hi16 (exact power-of-2
    scale cancellation; dropped lo*lo term is ~1e-4 relative).
  - DoubleRow contracts 2x128=256 in ONE instruction at 0.5 cyc/row, so
    each 3-term product costs 75% of the fp32r pair while keeping
    ~4e-3 end-to-end relative error (measured in sim_fp8.py).
  - Scale balance: scores operands q,k at sigma~0.354 (sqrt(1/8) each),
    qproj operands a*0.177 / M*2, vproj k_dev / wv_t/0.354.
  - AV stays fp32r with the ones-column denominator trick; numerator +
    denominator ship unnormalized to DRAM; host does divide + bias +
    transpose.
  - V projection in chunk pairs fused into group 0's attention loop.
"""

import numpy as np

B, C, HW = 4, 256, 4096
NQ = HW // 2          # queries per core
NCORES = 8
KC = HW // 128        # 32 key chunks
QG = NQ // 512        # 4 query groups of 512 per core
SCALE = 1.0 / 8.0     # 1/sqrt(dim_head=64)
SQ = float(np.sqrt(SCALE))

_COMPILED = {}


def _build_nc():
    import concourse.bass as bass
    from concourse import bacc, mybir
    import concourse.tile as tile

    dt = mybir.dt.float32
    rdt = mybir.dt.float32r
    e4 = mybir.dt.float8e4
    DR = mybir.MatmulPerfMode.DoubleRow
    Exp = mybir.ActivationFunctionType.Exp

    nc = bacc.Bacc("TRN2", target_bir_lowering=False, debug=False)

    # k/a packs: [128, 2, 3*N]: dim1 = channel half (ch = i*128+p),
    # dim2 = variant-major: [0:N]=hi, [N:2N]=lo16, [2N:3N]=hi16
    kp = nc.dram_tensor("kp", [128, 2, 3 * HW], e4, kind="ExternalInput")
    ap_d = nc.dram_tensor("ap", [128, 2, 3 * NQ], e4, kind="ExternalInput")
    # weight packs: [0:256]=wvh, [256:512]=wvl16, [512:768]=wvh16,
    #               [768:1024]=wqh, [1024:1280]=wql16, [1280:1536]=wqh16
    wp = nc.dram_tensor("wp", [128, 2, 1536], e4, kind="ExternalInput")
    bqd = nc.dram_tensor("bq", [C, 1], dt, kind="ExternalInput")
    out = nc.dram_tensor("out", [NQ, 258], dt, kind="ExternalOutput")

    with tile.TileContext(nc) as tc:
        with (
            tc.tile_pool(name="consts", bufs=1) as consts,
            tc.tile_pool(name="feat", bufs=1) as feat,
            tc.tile_pool(name="qkt", bufs=1) as qkt,
            tc.tile_pool(name="qpp", bufs=2) as qpp,
            tc.tile_pool(name="dqp", bufs=2) as dqp,
            tc.tile_pool(name="vsb", bufs=1) as vsb,
            tc.tile_pool(name="expp", bufs=4) as expp,
            tc.tile_pool(name="ostg", bufs=1) as ostg,
            tc.tile_pool(name="vp_ps", bufs=1, space="PSUM") as vp_ps,
            tc.tile_pool(name="s_ps", bufs=3, space="PSUM") as s_ps,
            tc.tile_pool(name="o_ps", bufs=1, space="PSUM") as o_ps,
        ):
            kp_sb = feat.tile([128, 2, 3 * HW], e4, tag="kp", name="kp")
            ap_sb = feat.tile([128, 2, 3 * NQ], e4, tag="ap", name="ap")
            wp_sb = consts.tile([128, 2, 1536], e4, tag="wp", name="wp")
            bq_sb = [consts.tile([128, 1], dt, tag=f"bq{j}", name=f"bq{j}")
                     for j in range(2)]
            v_sb = [vsb.tile([128, 258], rdt, tag=f"v{k}", name=f"v{k}")
                    for k in range(KC)]

            def kvar(v, c0, c1):
                return kp_sb[:, :, v * HW + c0:v * HW + c1]

            def avar(v, g):
                return ap_sb[:, :, v * NQ + g * 512:v * NQ + (g + 1) * 512]

            def wvar(idx):
                return wp_sb[:, :, idx * 256:(idx + 1) * 256]

            # ---- DMA program (all on SP/HWDGE, priority order) ----
            def dma_k(c0, c1):
                for v in range(3):
                    nc.sync.dma_start(out=kvar(v, c0, c1),
                                      in_=kp[:, :, v * HW + c0:v * HW + c1])

            nc.sync.dma_start(out=wp_sb, in_=wp[:, :, :])
            for j in range(2):
                nc.sync.dma_start(out=bq_sb[j],
                                  in_=bqd[j * 128:(j + 1) * 128, :])
            for v in range(3):
                nc.sync.dma_start(
                    out=ap_sb[:, :, v * NQ:v * NQ + 512],
                    in_=ap_d[:, :, v * NQ:v * NQ + 512])
            dma_k(0, 512)
            dma_k(512, 1536)
            for v in range(3):
                nc.sync.dma_start(
                    out=ap_sb[:, :, v * NQ + 512:(v + 1) * NQ],
                    in_=ap_d[:, :, v * NQ + 512:(v + 1) * NQ])
            dma_k(1536, 2560)
            dma_k(2560, 4096)

            # ones columns for the AV denominator; ACT table warm-up
            ones = consts.tile([128, 2], dt, tag="ones")
            nc.vector.memset(ones, 1.0)
            for k in range(KC):
                nc.vector.tensor_copy(v_sb[k][:, 256:258], ones)
            warm = consts.tile([128, 1], dt, tag="warm")
            nc.scalar.activation(out=warm, in_=ones[:, 0:1], func=Exp)

            # ---- emission helpers ----
            vcnt = [0]

            def vproj_pair(k):
                # chunks k, k+1 into one [128,512] PSUM tile; one copy out
                ps = vp_ps.tile([128, 512], dt, tag="vp", name=f"vp{k}")
                for kk in (k, k + 1):
                    sl = ps[:, (kk - k) * 256:(kk - k + 1) * 256]
                    nc.tensor.matmul(sl, kvar(0, kk * 128, (kk + 1) * 128),
                                     wvar(0), start=True, stop=False,
                                     perf_mode=DR)
                    nc.tensor.matmul(sl, kvar(2, kk * 128, (kk + 1) * 128),
                                     wvar(1), start=False, stop=False,
                                     perf_mode=DR)
                    nc.tensor.matmul(sl, kvar(1, kk * 128, (kk + 1) * 128),
                                     wvar(2), start=False, stop=True,
                                     perf_mode=DR)
                nc.vector.tensor_copy(v_sb[k][:, 0:256], ps[:, 0:256])
                nc.vector.tensor_copy(v_sb[k + 1][:, 0:256], ps[:, 256:512])

            qp = {}

            def qproj(g):
                # wq packs at wvar indices 3,4,5; lhsT free dim = do chunk.
                # q-prep (hi / lo16 / hi16 quantization) runs straight from
                # PSUM on DVE: hi = e4m3(ps + bq), dq = ps - hi,
                # lo16 = e4m3(16*dq + bq*16... bias already in hi), so
                # dq must subtract hi from the biased value: dq = (ps+bq)-hi.
                qph = qpp.tile([128, 2, 512], e4, tag="qph", name=f"qph{g}")
                qpl = qpp.tile([128, 2, 512], e4, tag="qpl", name=f"qpl{g}")
                qpu = qpp.tile([128, 2, 512], e4, tag="qpu", name=f"qpu{g}")
                for do in range(2):
                    ps = s_ps.tile([128, 512], dt, tag="sp", name=f"qp{g}{do}")
                    d0, d1 = do * 128, (do + 1) * 128
                    nc.tensor.matmul(ps, wvar(3)[:, :, d0:d1], avar(0, g),
                                     start=True, stop=False, perf_mode=DR)
                    nc.tensor.matmul(ps, wvar(5)[:, :, d0:d1], avar(1, g),
                                     start=False, stop=False, perf_mode=DR)
                    nc.tensor.matmul(ps, wvar(4)[:, :, d0:d1], avar(2, g),
                                     start=False, stop=True, perf_mode=DR)
                    hi = qph[:, do:do + 1, :]
                    qb = dqp.tile([128, 512], dt, tag="qb", name=f"qb{g}{do}")
                    dq = dqp.tile([128, 512], dt, tag="dq", name=f"dq{g}{do}")
                    if do == 0:
                        nc.vector.tensor_scalar_add(qb, ps, bq_sb[do])
                        nc.vector.tensor_copy(hi, qb)
                        nc.vector.tensor_sub(dq, qb, hi)
                        nc.vector.tensor_scalar_mul(qpl[:, do:do + 1, :], dq, 16.0)
                        nc.vector.tensor_scalar_mul(qpu[:, do:do + 1, :], hi, 0.0625)
                    else:
                        nc.scalar.add(qb, ps, bq_sb[do])
                        nc.scalar.copy(hi, qb)
                        nc.vector.tensor_sub(dq, qb, hi)
                        nc.vector.tensor_scalar_mul(qpl[:, do:do + 1, :], dq, 16.0)
                        nc.scalar.mul(qpu[:, do:do + 1, :], hi, 0.0625)
                qp[g] = (qph, qpl, qpu)

            ets = [None] * KC

            def scores(g, k):
                qph, qpl, qpu = qp[g]
                sp = s_ps.tile([128, 512], dt, tag="sp", name=f"s{g}_{k}")
                c0, c1 = k * 128, (k + 1) * 128
                nc.tensor.matmul(sp, kvar(0, c0, c1), qph,
                                 start=True, stop=False, perf_mode=DR)
                nc.tensor.matmul(sp, kvar(2, c0, c1), qpl,
                                 start=False, stop=False, perf_mode=DR)
                nc.tensor.matmul(sp, kvar(1, c0, c1), qpu,
                                 start=False, stop=True, perf_mode=DR)
                et = expp.tile([128, 512], rdt, tag="et", name=f"e{g}_{k}")
                nc.scalar.activation(out=et, in_=sp, func=Exp)
                ets[k] = et

            o_acc = [o_ps.tile([128, 258], dt, tag=f"o{qs}", name=f"o{qs}")
                     for qs in range(4)]

            def av(g, k):
                for qs in range(4):
                    nc.tensor.matmul(
                        o_acc[qs],
                        ets[k][:, qs * 128:(qs + 1) * 128],
                        v_sb[k][:, 0:258],
                        start=(k == 0), stop=(k == KC - 1),
                    )
                ets[k] = None

            def flush(g):
                for qs in range(4):
                    ot = ostg.tile([128, 258], dt, tag=f"ot{qs}",
                                   name=f"ot{g}{qs}")
                    if g == QG - 1 and qs % 2 == 1:
                        nc.scalar.copy(ot, o_acc[qs])
                    else:
                        nc.vector.tensor_copy(ot, o_acc[qs])
                    # split store issue between HWDGE (SP) and SWDGE (Pool)
                    eng = nc.sync if qs % 2 == 0 else nc.gpsimd
                    eng.dma_start(
                        out=out[(g * 4 + qs) * 128:(g * 4 + qs + 1) * 128, :],
                        in_=ot)

            # ---- schedule ----
            for k in range(0, 4, 2):
                vproj_pair(k)
            # (qproj(0) below; remaining vproj pairs fused into group 0)
            qproj(0)
            # group 0 (fused remaining vproj pairs), 2-deep scores pipeline
            next_v = [4]

            def maybe_vproj():
                if next_v[0] < KC:
                    vproj_pair(next_v[0])
                    next_v[0] += 2

            for g in range(QG):
                scores(g, 0)
                scores(g, 1)
                for k in range(KC):
                    if k >= 1:
                        av(g, k - 1)
                    if k + 2 < KC:
                        scores(g, k + 2)
                    if g == 0 and k % 2 == 1:
                        maybe_vproj()
                    if k == 16 and g < QG - 1:
                        qproj(g + 1)
                av(g, KC - 1)
                flush(g)
    nc.finalize()
    return nc


def _get_nc():
    if "nc" not in _COMPILED:
        _COMPILED["nc"] = _build_nc()
    return _COMPILED["nc"]


def _get_runner():
    """Jit the SPMD executable once and reuse it across kernel() calls."""
    if "runner" in _COMPILED:
        return _COMPILED["runner"]
    import jax
    from jax.experimental.shard_map import shard_map
    from jax.sharding import Mesh, PartitionSpec
    from concourse import bass2jax, mybir
    from concourse.bass2jax import _bass_exec_p, install_neuronx_cc_hook

    nc = _get_nc()
    install_neuronx_cc_hook()
    try:
        jax.config.update("jax_compilation_cache_dir", "/tmp/jax_cache")
        jax.config.update("jax_persistent_cache_min_compile_time_secs", 0.0)
        jax.config.update("jax_persistent_cache_min_entry_size_bytes", -1)
    except Exception:
        pass
    in_names, out_names, out_avals, zero_outs = [], [], [], []
    for alloc in nc.m.functions[0].allocations:
        if not isinstance(alloc, mybir.MemoryLocationSet):
            continue
        name = alloc.memorylocations[0].name
        if alloc.kind == "ExternalInput":
            if nc.partition_id_tensor is None or \
                    name != nc.partition_id_tensor.name:
                in_names.append(name)
        elif alloc.kind == "ExternalOutput":
            out_names.append(name)
            shape = tuple(alloc.tensor_shape)
            dtype = mybir.dt.np(alloc.dtype)
            out_avals.append(jax.core.ShapedArray(shape, dtype))
            zero_outs.append(np.zeros(shape, dtype))
    all_names = in_names + out_names
    if nc.partition_id_tensor is not None:
        all_names.append(nc.partition_id_tensor.name)

    def _body(*args):
        operands = list(args)
        if nc.partition_id_tensor is not None:
            operands.append(bass2jax.partition_id_tensor())
        return tuple(_bass_exec_p.bind(
            *operands, out_avals=tuple(out_avals), in_names=tuple(all_names),
            out_names=tuple(out_names), lowering_input_output_aliases=(),
            sim_require_finite=True, sim_require_nnan=True, nc=nc))

    devices = jax.devices()[:NCORES]
    mesh = Mesh(np.asarray(devices), ("core",))
    n_io = len(in_names) + len(out_names)
    sharded = jax.jit(
        shard_map(_body, mesh=mesh,
                  in_specs=(PartitionSpec("core"),) * n_io,
                  out_specs=(PartitionSpec("core"),) * len(out_names),
                  check_rep=False),
        keep_unused=True)
    _COMPILED["runner"] = (sharded, in_names, out_names, zero_outs)
    return _COMPILED["runner"]


def _split_pack(x):
    """x [128, 2, N] f32 -> concat([hi, lo16, hi16], axis=2) in e4m3."""
    import ml_dtypes
    E4 = ml_dtypes.float8_e4m3
    hi = x.astype(E4)
    hif = hi.astype(np.float32)
    lo16 = ((x - hif) * np.float32(16.0)).astype(E4)
    hi16 = (hif * np.float32(0.0625)).astype(E4)
    return np.concatenate([hi, lo16, hi16], axis=2)


def _chpack(x):
    """[C, N] -> [128, 2, N]: channel ch=i*128+p -> (p, i)."""
    return np.ascontiguousarray(
        x.reshape(2, 128, x.shape[1]).transpose(1, 0, 2))


def kernel(feat_A, feat_B, Wq, bq, Wk, bk, Wv, bv, Wo, bo, **_unused):
    f32 = np.float32
    fa = np.asarray(feat_A, f32).reshape(B, C, HW)
    fb = np.asarray(feat_B, f32).reshape(B, C, HW)
    # fold Wk into the Q projection and Wo into the V projection; the
    # (Q-bias . bk) cross term is a per-query constant, which softmax
    # ignores, so it is dropped exactly.
    Wq64 = np.asarray(Wq, np.float64) * SCALE
    Wk64 = np.asarray(Wk, np.float64)
    Wv64 = np.asarray(Wv, np.float64)
    Wo64 = np.asarray(Wo, np.float64)
    M = (Wq64.T @ Wk64) / SCALE                     # Wq^T Wk (unscaled)
    wv_t = (Wo64 @ Wv64).T                          # [C(in), C(d)]
    bq_s = (np.asarray(bq, np.float64) @ Wk64)
    bv_r = (Wo64 @ np.asarray(bv, np.float64)).astype(f32)
    bo_c = np.asarray(bo, f32)

    # device scaling: q_dev = (a*SA) @ (M*2) + bq*SQ  (sigma ~0.354)
    #                 k_dev = b*SQ;  v = k_dev^T @ (wv_t/SQ)
    SA = f32(SQ / 2)
    wq_dev = (M * 2.0).astype(f32)
    bq_dev = (bq_s * SQ).astype(f32).reshape(C, 1)
    wv_dev = (wv_t / SQ).astype(f32)

    wpack = np.concatenate([
        _split_pack(_chpack(wv_dev)),
        _split_pack(_chpack(wq_dev)),
    ], axis=2)

    # k packs are shared by the two cores of each batch
    kpacks = [_split_pack(_chpack((fb[b] * f32(SQ)))) for b in range(B)]

    in_maps = []
    for cidx in range(NCORES):
        b, qh = cidx // 2, cidx % 2
        a_half = fa[b][:, qh * NQ:(qh + 1) * NQ] * SA
        in_maps.append({
            "kp": kpacks[b],
            "ap": _split_pack(_chpack(a_half)),
            "wp": wpack,
            "bq": bq_dev,
        })

    try:
        sharded, in_names, out_names, zero_outs = _get_runner()
        concat_in = [np.concatenate([in_maps[c][nm] for c in range(NCORES)],
                                    axis=0) for nm in in_names]
        concat_zeros = [np.zeros((NCORES * z.shape[0], *z.shape[1:]), z.dtype)
                        for z in zero_outs]
        out_arrs = sharded(*concat_in, *concat_zeros)
        res_out = np.asarray(out_arrs[out_names.index("out")]) \
            .reshape(NCORES, NQ, 258)
    except Exception:
        from concourse.bass_utils import run_bass_kernel_spmd
        res = run_bass_kernel_spmd(_get_nc(), in_maps, list(range(NCORES)))
        res_out = np.stack([res.results[c]["out"] for c in range(NCORES)])

    add_c = (bv_r + bo_c).astype(f32)               # [C]
    outf = np.empty((B, C, HW), f32)
    for cidx in range(NCORES):
        b, qh = cidx // 2, cidx % 2
        num = res_out[cidx][:, 0:256]
        den = res_out[cidx][:, 256:257]
        tok = num / den + add_c                     # [NQ, C]
        outf[b][:, qh * NQ:(qh + 1) * NQ] = tok.T
    return outf.reshape(B, C, 64, 64)


if __name__ == "__main__":
    rng = np.random.default_rng(0)
    ins = {
        "feat_A": rng.standard_normal((B, C, 64, 64), dtype=np.float32),
        "feat_B": rng.standard_normal((B, C, 64, 64), dtype=np.float32),
    }
    for nm in ("q", "k", "v", "o"):
        ins[f"W{nm}"] = rng.standard_normal((C, C), dtype=np.float32) / 16.0
        ins[f"b{nm}"] = np.zeros(C, np.float32)
    o = kernel(**ins)
    print("kernel ran, out shape", o.shape, "mean", float(np.abs(o).mean()))


# revision 12
# speedup vs baseline: 1.1642x; 1.0095x over previous
"""Cross-attention Trainium2 kernel (8 NeuronCores, SPMD).

Problem: B=4, C=256, H=W=64 -> N=4096 tokens/batch, single-head attention
over full C=256 with scale 1/sqrt(64)=1/8, then output projection.

Sharding: 2 cores per batch; each core owns 2048 queries (half the batch's
4096) and replicates K/V work for its batch (cheap vs. collectives).

v4 strategy: split-precision fp8 DoubleRow matmuls for scores and both
projections; fp32r for AV.

  - Wk folded into Q projection on host, Wo folded into Wv.
  - Every fp8 operand x ships as three e4m3 tensors: hi=fp8(x),
    lo16=fp8((x-hi)*16), hi16=fp8(hi/16).  A product x@y is computed as
    3 DoubleRow matmuls: hi@hi + hi16@lo16 + lo16@hi16 (exact power-of-2
    scale cancellation; dropped lo*lo term is ~1e-4 relative).
  - DoubleRow contracts 2x128=256 in ONE instruction at 0.5 cyc/row, so
    each 3-term product costs 75% of the fp32r pair while keeping
    ~4e-3 end-to-end relative error (measured in sim_fp8.py).
  - Scale balance: scores operands q,k at sigma~0.354 (sqrt(1/8) each),
    qproj operands a*0.177 / M*2, vproj k_dev / wv_t/0.354.
  - AV stays fp32r with the ones-column denominator trick; numerator +
    denominator ship unnormalized to DRAM; host does divide + bias +
    transpose.
  - V projection in chunk pairs fused into group 0's attention loop.
"""

import numpy as np

B, C, HW = 4, 256, 4096
NQ = HW // 2          # queries per core
NCORES = 8
KC = HW // 128        # 32 key chunks
QG = NQ // 512        # 4 query groups of 512 per core
SCALE = 1.0 / 8.0     # 1/sqrt(dim_head=64)
SQ = float(np.sqrt(SCALE))

_COMPILED = {}


def _build_nc():
    import concourse.bass as bass
    from concourse import bacc, mybir
    import concourse.tile as tile

    dt = mybir.dt.float32
    rdt = mybir.dt.float32r
    e4 = mybir.dt.float8e4
    DR = mybir.MatmulPerfMode.DoubleRow
    Exp = mybir.ActivationFunctionType.Exp

    nc = bacc.Bacc("TRN2", target_bir_lowering=False, debug=False)

    # k/a packs: [128, 2, 3*N]: dim1 = channel half (ch = i*128+p),
    # dim2 = variant-major: [0:N]=hi, [N:2N]=lo16, [2N:3N]=hi16
    kp = nc.dram_tensor("kp", [128, 2, 3 * HW], e4, kind="ExternalInput")
    ap_d = nc.dram_tensor("ap", [128, 2, 3 * NQ], e4, kind="ExternalInput")
    # weight packs: [0:256]=wvh, [256:512]=wvl16, [512:768]=wvh16,
    #               [768:1024]=wqh, [1024:1280]=wql16, [1280:1536]=wqh16
    wp = nc.dram_tensor("wp", [128, 2, 1536], e4, kind="ExternalInput")
    bqd = nc.dram_tensor("bq", [128, 2], dt, kind="ExternalInput")
    out = nc.dram_tensor("out", [NQ, 258], dt, kind="ExternalOutput")

    with tile.TileContext(nc) as tc:
        with (
            tc.tile_pool(name="consts", bufs=1) as consts,
            tc.tile_pool(name="feat", bufs=1) as feat,
            tc.tile_pool(name="qkt", bufs=1) as qkt,
            tc.tile_pool(name="qpp", bufs=2) as qpp,
            tc.tile_pool(name="dqp", bufs=2) as dqp,
            tc.tile_pool(name="vsb", bufs=1) as vsb,
            tc.tile_pool(name="expp", bufs=4) as expp,
            tc.tile_pool(name="ostg", bufs=1) as ostg,
            tc.tile_pool(name="vp_ps", bufs=1, space="PSUM") as vp_ps,
            tc.tile_pool(name="s_ps", bufs=3, space="PSUM") as s_ps,
            tc.tile_pool(name="o_ps", bufs=1, space="PSUM") as o_ps,
        ):
            kp_sb = feat.tile([128, 2, 3 * HW], e4, tag="kp", name="kp")
            ap_sb = feat.tile([128, 2, 3 * NQ], e4, tag="ap", name="ap")
            wp_sb = consts.tile([128, 2, 1536], e4, tag="wp", name="wp")
            bq_sb = consts.tile([128, 2], dt, tag="bq", name="bq")
            v_sb = [vsb.tile([128, 258], rdt, tag=f"v{k}", name=f"v{k}")
                    for k in range(KC)]

            def kvar(v, c0, c1):
                return kp_sb[:, :, v * HW + c0:v * HW + c1]

            def avar(v, g):
                return ap_sb[:, :, v * NQ + g * 512:v * NQ + (g + 1) * 512]

            def wvar(idx):
                return wp_sb[:, :, idx * 256:(idx + 1) * 256]

            # ---- DMA program (all on SP/HWDGE, priority order) ----
            def dma_k(c0, c1):
                for v in range(3):
                    nc.sync.dma_start(out=kvar(v, c0, c1),
                                      in_=kp[:, :, v * HW + c0:v * HW + c1])

            nc.sync.dma_start(out=wp_sb, in_=wp[:, :, :])
            nc.sync.dma_start(out=bq_sb, in_=bqd[:, :])
            for v in range(3):
                nc.sync.dma_start(
                    out=ap_sb[:, :, v * NQ:v * NQ + 512],
                    in_=ap_d[:, :, v * NQ:v * NQ + 512])
            dma_k(0, 512)
            dma_k(512, 1536)
            dma_k(1536, 2560)
            for v in range(3):
                nc.sync.dma_start(
                    out=ap_sb[:, :, v * NQ + 512:(v + 1) * NQ],
                    in_=ap_d[:, :, v * NQ + 512:(v + 1) * NQ])
            dma_k(2560, 4096)

            # ones columns for the AV denominator; ACT table warm-up
            ones = consts.tile([128, 2], dt, tag="ones")
            nc.vector.memset(ones, 1.0)
            for k in range(KC):
                nc.vector.tensor_copy(v_sb[k][:, 256:258], ones)
            warm = consts.tile([128, 1], dt, tag="warm")
            nc.scalar.activation(out=warm, in_=ones[:, 0:1], func=Exp)

            # ---- emission helpers ----
            vcnt = [0]

            def vproj_pair(k):
                # chunks k, k+1 into one [128,512] PSUM tile; one copy out
                ps = vp_ps.tile([128, 512], dt, tag="vp", name=f"vp{k}")
                for kk in (k, k + 1):
                    sl = ps[:, (kk - k) * 256:(kk - k + 1) * 256]
                    nc.tensor.matmul(sl, kvar(0, kk * 128, (kk + 1) * 128),
                                     wvar(0), start=True, stop=False,
                                     perf_mode=DR)
                    nc.tensor.matmul(sl, kvar(2, kk * 128, (kk + 1) * 128),
                                     wvar(1), start=False, stop=False,
                                     perf_mode=DR)
                    nc.tensor.matmul(sl, kvar(1, kk * 128, (kk + 1) * 128),
                                     wvar(2), start=False, stop=True,
                                     perf_mode=DR)
                nc.vector.tensor_copy(v_sb[k][:, 0:256], ps[:, 0:256])
                nc.vector.tensor_copy(v_sb[k + 1][:, 0:256], ps[:, 256:512])

            qp = {}

            def qproj(g):
                # wq packs at wvar indices 3,4,5; lhsT free dim = do chunk.
                # q-prep (hi / lo16 / hi16 quantization) runs straight from
                # PSUM on DVE: hi = e4m3(ps + bq), dq = ps - hi,
                # lo16 = e4m3(16*dq + bq*16... bias already in hi), so
                # dq must subtract hi from the biased value: dq = (ps+bq)-hi.
                qph = qpp.tile([128, 2, 512], e4, tag="qph", name=f"qph{g}")
                qpl = qpp.tile([128, 2, 512], e4, tag="qpl", name=f"qpl{g}")
                qpu = qpp.tile([128, 2, 512], e4, tag="qpu", name=f"qpu{g}")
                for do in range(2):
                    ps = s_ps.tile([128, 512], dt, tag="sp", name=f"qp{g}{do}")
                    d0, d1 = do * 128, (do + 1) * 128
                    nc.tensor.matmul(ps, wvar(3)[:, :, d0:d1], avar(0, g),
                                     start=True, stop=False, perf_mode=DR)
                    nc.tensor.matmul(ps, wvar(5)[:, :, d0:d1], avar(1, g),
                                     start=False, stop=False, perf_mode=DR)
                    nc.tensor.matmul(ps, wvar(4)[:, :, d0:d1], avar(2, g),
                                     start=False, stop=True, perf_mode=DR)
                    hi = qph[:, do:do + 1, :]
                    qb = dqp.tile([128, 512], dt, tag="qb", name=f"qb{g}{do}")
                    dq = dqp.tile([128, 512], dt, tag="dq", name=f"dq{g}{do}")
                    if do == 0:
                        nc.vector.tensor_scalar_add(qb, ps, bq_sb[:, do:do + 1])
                        nc.vector.tensor_copy(hi, qb)
                        nc.vector.tensor_sub(dq, qb, hi)
                        nc.vector.tensor_scalar_mul(qpl[:, do:do + 1, :], dq, 16.0)
                        nc.vector.tensor_scalar_mul(qpu[:, do:do + 1, :], hi, 0.0625)
                    else:
                        nc.scalar.add(qb, ps, bq_sb[:, do:do + 1])
                        nc.scalar.copy(hi, qb)
                        nc.vector.tensor_sub(dq, qb, hi)
                        nc.vector.tensor_scalar_mul(qpl[:, do:do + 1, :], dq, 16.0)
                        nc.scalar.mul(qpu[:, do:do + 1, :], hi, 0.0625)
                qp[g] = (qph, qpl, qpu)

            ets = [None] * KC

            def scores(g, k):
                qph, qpl, qpu = qp[g]
                sp = s_ps.tile([128, 512], dt, tag="sp", name=f"s{g}_{k}")
                c0, c1 = k * 128, (k + 1) * 128
                nc.tensor.matmul(sp, kvar(0, c0, c1), qph,
                                 start=True, stop=False, perf_mode=DR)
                nc.tensor.matmul(sp, kvar(2, c0, c1), qpl,
                                 start=False, stop=False, perf_mode=DR)
                nc.tensor.matmul(sp, kvar(1, c0, c1), qpu,
                                 start=False, stop=True, perf_mode=DR)
                et = expp.tile([128, 512], rdt, tag="et", name=f"e{g}_{k}")
                nc.scalar.activation(out=et, in_=sp, func=Exp)
                ets[k] = et

            o_acc = [o_ps.tile([128, 258], dt, tag=f"o{qs}", name=f"o{qs}")
                     for qs in range(4)]

            def av(g, k):
                order = range(4)
                if g == QG - 1 and k == KC - 1:
                    order = range(3, -1, -1)
                for qs in order:
                    nc.tensor.matmul(
                        o_acc[qs],
                        ets[k][:, qs * 128:(qs + 1) * 128],
                        v_sb[k][:, 0:258],
                        start=(k == 0), stop=(k == KC - 1),
                    )
                ets[k] = None

            def flush(g):
                last = g == QG - 1
                order = range(3, -1, -1) if last else range(4)
                for n, qs in enumerate(order):
                    ot = ostg.tile([128, 258], dt, tag=f"ot{qs}",
                                   name=f"ot{g}{qs}")
                    if last and n % 2 == 1:
                        nc.scalar.copy(ot, o_acc[qs])
                    else:
                        nc.vector.tensor_copy(ot, o_acc[qs])
                    # split store issue between HWDGE (SP) and SWDGE (Pool)
                    eng = nc.sync if n % 2 == 0 else nc.gpsimd
                    eng.dma_start(
                        out=out[(g * 4 + qs) * 128:(g * 4 + qs + 1) * 128, :],
                        in_=ot)

            # ---- schedule ----
            qproj(0)
            for k in range(0, 4, 2):
                vproj_pair(k)
            # group 0 (fused remaining vproj pairs), 2-deep scores pipeline
            next_v = [4]

            def maybe_vproj():
                if next_v[0] < KC:
                    vproj_pair(next_v[0])
                    next_v[0] += 2

            for g in range(QG):
                scores(g, 0)
                scores(g, 1)
                for k in range(KC):
                    if k >= 1:
                        av(g, k - 1)
                    if k + 2 < KC:
                        scores(g, k + 2)
                    if g == 0 and k % 2 == 1:
                        maybe_vproj()
                    if k == 16 and g < QG - 1:
                        qproj(g + 1)
                av(g, KC - 1)
                flush(g)
    nc.finalize()
    return nc


def _get_nc():
    if "nc" not in _COMPILED:
        _COMPILED["nc"] = _build_nc()
    return _COMPILED["nc"]


def _get_runner():
    """Jit the SPMD executable once and reuse it across kernel() calls."""
    if "runner" in _COMPILED:
        return _COMPILED["runner"]
    import jax
    from jax.experimental.shard_map import shard_map
    from jax.sharding import Mesh, PartitionSpec
    from concourse import bass2jax, mybir
    from concourse.bass2jax import _bass_exec_p, install_neuronx_cc_hook

    nc = _get_nc()
    install_neuronx_cc_hook()
    try:
        jax.config.update("jax_compilation_cache_dir", "/tmp/jax_cache")
        jax.config.update("jax_persistent_cache_min_compile_time_secs", 0.0)
        jax.config.update("jax_persistent_cache_min_entry_size_bytes", -1)
    except Exception:
        pass
    in_names, out_names, out_avals, zero_outs = [], [], [], []
    for alloc in nc.m.functions[0].allocations:
        if not isinstance(alloc, mybir.MemoryLocationSet):
            continue
        name = alloc.memorylocations[0].name
        if alloc.kind == "ExternalInput":
            if nc.partition_id_tensor is None or \
                    name != nc.partition_id_tensor.name:
                in_names.append(name)
        elif alloc.kind == "ExternalOutput":
            out_names.append(name)
            shape = tuple(alloc.tensor_shape)
            dtype = mybir.dt.np(alloc.dtype)
            out_avals.append(jax.core.ShapedArray(shape, dtype))
            zero_outs.append(np.zeros(shape, dtype))
    all_names = in_names + out_names
    if nc.partition_id_tensor is not None:
        all_names.append(nc.partition_id_tensor.name)

    def _body(*args):
        operands = list(args)
        if nc.partition_id_tensor is not None:
            operands.append(bass2jax.partition_id_tensor())
        return tuple(_bass_exec_p.bind(
            *operands, out_avals=tuple(out_avals), in_names=tuple(all_names),
            out_names=tuple(out_names), lowering_input_output_aliases=(),
            sim_require_finite=True, sim_require_nnan=True, nc=nc))

    devices = jax.devices()[:NCORES]
    mesh = Mesh(np.asarray(devices), ("core",))
    n_io = len(in_names) + len(out_names)
    sharded = jax.jit(
        shard_map(_body, mesh=mesh,
                  in_specs=(PartitionSpec("core"),) * n_io,
                  out_specs=(PartitionSpec("core"),) * len(out_names),
                  check_rep=False),
        keep_unused=True)
    _COMPILED["runner"] = (sharded, in_names, out_names, zero_outs)
    return _COMPILED["runner"]


def _split_pack(x):
    """x [128, 2, N] f32 -> concat([hi, lo16, hi16], axis=2) in e4m3."""
    import ml_dtypes
    E4 = ml_dtypes.float8_e4m3
    hi = x.astype(E4)
    hif = hi.astype(np.float32)
    lo16 = ((x - hif) * np.float32(16.0)).astype(E4)
    hi16 = (hif * np.float32(0.0625)).astype(E4)
    return np.concatenate([hi, lo16, hi16], axis=2)


def _chpack(x):
    """[C, N] -> [128, 2, N]: channel ch=i*128+p -> (p, i)."""
    return np.ascontiguousarray(
        x.reshape(2, 128, x.shape[1]).transpose(1, 0, 2))


def kernel(feat_A, feat_B, Wq, bq, Wk, bk, Wv, bv, Wo, bo, **_unused):
    f32 = np.float32
    fa = np.asarray(feat_A, f32).reshape(B, C, HW)
    fb = np.asarray(feat_B, f32).reshape(B, C, HW)
    # fold Wk into the Q projection and Wo into the V projection; the
    # (Q-bias . bk) cross term is a per-query constant, which softmax
    # ignores, so it is dropped exactly.
    Wq64 = np.asarray(Wq, np.float64) * SCALE
    Wk64 = np.asarray(Wk, np.float64)
    Wv64 = np.asarray(Wv, np.float64)
    Wo64 = np.asarray(Wo, np.float64)
    M = (Wq64.T @ Wk64) / SCALE                     # Wq^T Wk (unscaled)
    wv_t = (Wo64 @ Wv64).T                          # [C(in), C(d)]
    bq_s = (np.asarray(bq, np.float64) @ Wk64)
    bv_r = (Wo64 @ np.asarray(bv, np.float64)).astype(f32)
    bo_c = np.asarray(bo, f32)

    # device scaling: q_dev = (a*SA) @ (M*2) + bq*SQ  (sigma ~0.354)
    #                 k_dev = b*SQ;  v = k_dev^T @ (wv_t/SQ)
    SA = f32(SQ / 2)
    wq_dev = (M * 2.0).astype(f32)
    bq_dev = np.ascontiguousarray(
        (bq_s * SQ).astype(f32).reshape(2, 128).T)
    wv_dev = (wv_t / SQ).astype(f32)

    wpack = np.concatenate([
        _split_pack(_chpack(wv_dev)),
        _split_pack(_chpack(wq_dev)),
    ], axis=2)

    # k packs are shared by the two cores of each batch
    kpacks = [_split_pack(_chpack((fb[b] * f32(SQ)))) for b in range(B)]

    in_maps = []
    for cidx in range(NCORES):
        b, qh = cidx // 2, cidx % 2
        a_half = fa[b][:, qh * NQ:(qh + 1) * NQ] * SA
        in_maps.append({
            "kp": kpacks[b],
            "ap": _split_pack(_chpack(a_half)),
            "wp": wpack,
            "bq": bq_dev,
        })

    try:
        sharded, in_names, out_names, zero_outs = _get_runner()
        concat_in = [np.concatenate([in_maps[c][nm] for c in range(NCORES)],
                                    axis=0) for nm in in_names]
        concat_zeros = [np.zeros((NCORES * z.shape[0], *z.shape[1:]), z.dtype)
                        for z in zero_outs]
        out_arrs = sharded(*concat_in, *concat_zeros)
        res_out = np.asarray(out_arrs[out_names.index("out")]) \
            .reshape(NCORES, NQ, 258)
    except Exception:
        from concourse.bass_utils import run_bass_kernel_spmd
        res = run_bass_kernel_spmd(_get_nc(), in_maps, list(range(NCORES)))
        res_out = np.stack([res.results[c]["out"] for c in range(NCORES)])

    add_c = (bv_r + bo_c).astype(f32)               # [C]
    outf = np.empty((B, C, HW), f32)
    for cidx in range(NCORES):
        b, qh = cidx // 2, cidx % 2
        num = res_out[cidx][:, 0:256]
        den = res_out[cidx][:, 256:257]
        tok = num / den + add_c                     # [NQ, C]
        outf[b][:, qh * NQ:(qh + 1) * NQ] = tok.T
    return outf.reshape(B, C, 64, 64)


if __name__ == "__main__":
    rng = np.random.default_rng(0)
    ins = {
        "feat_A": rng.standard_normal((B, C, 64, 64), dtype=np.float32),
        "feat_B": rng.standard_normal((B, C, 64, 64), dtype=np.float32),
    }
    for nm in ("q", "k", "v", "o"):
        ins[f"W{nm}"] = rng.standard_normal((C, C), dtype=np.float32) / 16.0
        ins[f"b{nm}"] = np.zeros(C, np.float32)
    o = kernel(**ins)
    print("kernel ran, out shape", o.shape, "mean", float(np.abs(o).mean()))


# revision 13
# speedup vs baseline: 1.1818x; 1.0151x over previous
"""Cross-attention Trainium2 kernel (8 NeuronCores, SPMD).

Problem: B=4, C=256, H=W=64 -> N=4096 tokens/batch, single-head attention
over full C=256 with scale 1/sqrt(64)=1/8, then output projection.

Sharding: 2 cores per batch; each core owns 2048 queries (half the batch's
4096) and replicates K/V work for its batch (cheap vs. collectives).

v4 strategy: split-precision fp8 DoubleRow matmuls for scores and both
projections; fp32r for AV.

  - Wk folded into Q projection on host, Wo folded into Wv.
  - Every fp8 operand x ships as three e4m3 tensors: hi=fp8(x),
    lo16=fp8((x-hi)*16), hi16=fp8(hi/16).  A product x@y is computed as
    3 DoubleRow matmuls: hi@hi + hi16@lo16 + lo16@hi16 (exact power-of-2
    scale cancellation; dropped lo*lo term is ~1e-4 relative).
  - DoubleRow contracts 2x128=256 in ONE instruction at 0.5 cyc/row, so
    each 3-term product costs 75% of the fp32r pair while keeping
    ~4e-3 end-to-end relative error (measured in sim_fp8.py).
  - Scale balance: scores operands q,k at sigma~0.354 (sqrt(1/8) each),
    qproj operands a*0.177 / M*2, vproj k_dev / wv_t/0.354.
  - AV stays fp32r with the ones-column denominator trick; numerator +
    denominator ship unnormalized to DRAM; host does divide + bias +
    transpose.
  - V projection in chunk pairs fused into group 0's attention loop.
"""

import numpy as np

B, C, HW = 4, 256, 4096
NQ = HW // 2          # queries per core
NCORES = 8
KC = HW // 128        # 32 key chunks
QG = NQ // 512        # 4 query groups of 512 per core
SCALE = 1.0 / 8.0     # 1/sqrt(dim_head=64)
SQ = float(np.sqrt(SCALE))

_COMPILED = {}


def _build_nc():
    import concourse.bass as bass
    from concourse import bacc, mybir
    import concourse.tile as tile

    dt = mybir.dt.float32
    rdt = mybir.dt.float32r
    e4 = mybir.dt.float8e4
    DR = mybir.MatmulPerfMode.DoubleRow
    Exp = mybir.ActivationFunctionType.Exp

    nc = bacc.Bacc("TRN2", target_bir_lowering=False, debug=False)

    # k/a packs: [128, 2, 3*N]: dim1 = channel half (ch = i*128+p),
    # dim2 = variant-major: [0:N]=hi, [N:2N]=lo16, [2N:3N]=hi16
    kp = nc.dram_tensor("kp", [128, 2, 3 * HW], e4, kind="ExternalInput")
    ap_d = nc.dram_tensor("ap", [128, 2, 3 * NQ], e4, kind="ExternalInput")
    # weight packs: [0:256]=wvh, [256:512]=wvl16, [512:768]=wvh16,
    #               [768:1024]=wqh, [1024:1280]=wql16, [1280:1536]=wqh16
    wp = nc.dram_tensor("wp", [128, 2, 1536], e4, kind="ExternalInput")
    bqd = nc.dram_tensor("bq", [128, 2], dt, kind="ExternalInput")
    out = nc.dram_tensor("out", [NQ, 258], dt, kind="ExternalOutput")

    with tile.TileContext(nc) as tc:
        with (
            tc.tile_pool(name="consts", bufs=1) as consts,
            tc.tile_pool(name="feat", bufs=1) as feat,
            tc.tile_pool(name="qkt", bufs=1) as qkt,
            tc.tile_pool(name="qpp", bufs=2) as qpp,
            tc.tile_pool(name="dqp", bufs=2) as dqp,
            tc.tile_pool(name="vsb", bufs=1) as vsb,
            tc.tile_pool(name="expp", bufs=4) as expp,
            tc.tile_pool(name="ostg", bufs=1) as ostg,
            tc.tile_pool(name="vp_ps", bufs=1, space="PSUM") as vp_ps,
            tc.tile_pool(name="s_ps", bufs=3, space="PSUM") as s_ps,
            tc.tile_pool(name="o_ps", bufs=1, space="PSUM") as o_ps,
        ):
            kp_sb = feat.tile([128, 2, 3 * HW], e4, tag="kp", name="kp")
            ap_sb = feat.tile([128, 2, 3 * NQ], e4, tag="ap", name="ap")
            wp_sb = consts.tile([128, 2, 1536], e4, tag="wp", name="wp")
            bq_sb = consts.tile([128, 2], dt, tag="bq", name="bq")
            v_sb = [vsb.tile([128, 258], rdt, tag=f"v{k}", name=f"v{k}")
                    for k in range(KC)]

            def kvar(v, c0, c1):
                return kp_sb[:, :, v * HW + c0:v * HW + c1]

            def avar(v, g):
                return ap_sb[:, :, v * NQ + g * 512:v * NQ + (g + 1) * 512]

            def wvar(idx):
                return wp_sb[:, :, idx * 256:(idx + 1) * 256]

            # ---- DMA program (all on SP/HWDGE, priority order) ----
            def dma_k(c0, c1):
                for v in range(3):
                    nc.sync.dma_start(out=kvar(v, c0, c1),
                                      in_=kp[:, :, v * HW + c0:v * HW + c1])

            nc.sync.dma_start(out=wp_sb, in_=wp[:, :, :])
            nc.sync.dma_start(out=bq_sb, in_=bqd[:, :])
            for v in range(3):
                nc.sync.dma_start(
                    out=ap_sb[:, :, v * NQ:v * NQ + 512],
                    in_=ap_d[:, :, v * NQ:v * NQ + 512])
            dma_k(0, 512)
            dma_k(512, 1536)
            dma_k(1536, 2560)
            for v in range(3):
                nc.sync.dma_start(
                    out=ap_sb[:, :, v * NQ + 512:(v + 1) * NQ],
                    in_=ap_d[:, :, v * NQ + 512:(v + 1) * NQ])
            dma_k(2560, 4096)

            # ones columns for the AV denominator; ACT table warm-up
            ones = consts.tile([128, 2], dt, tag="ones")
            nc.gpsimd.memset(ones, 1.0)
            for k in range(KC):
                nc.gpsimd.tensor_copy(v_sb[k][:, 256:258], ones)
            warm = consts.tile([128, 1], dt, tag="warm")
            nc.scalar.activation(out=warm, in_=ones[:, 0:1], func=Exp)

            # ---- emission helpers ----
            vcnt = [0]

            def vproj_pair(k):
                # chunks k, k+1 into one [128,512] PSUM tile; one copy out
                ps = vp_ps.tile([128, 512], dt, tag="vp", name=f"vp{k}")
                for kk in (k, k + 1):
                    sl = ps[:, (kk - k) * 256:(kk - k + 1) * 256]
                    nc.tensor.matmul(sl, kvar(0, kk * 128, (kk + 1) * 128),
                                     wvar(0), start=True, stop=False,
                                     perf_mode=DR)
                    nc.tensor.matmul(sl, kvar(2, kk * 128, (kk + 1) * 128),
                                     wvar(1), start=False, stop=False,
                                     perf_mode=DR)
                    nc.tensor.matmul(sl, kvar(1, kk * 128, (kk + 1) * 128),
                                     wvar(2), start=False, stop=True,
                                     perf_mode=DR)
                if k < 4:
                    nc.scalar.copy(v_sb[k][:, 0:256], ps[:, 0:256])
                    nc.scalar.copy(v_sb[k + 1][:, 0:256], ps[:, 256:512])
                else:
                    nc.vector.tensor_copy(v_sb[k][:, 0:256], ps[:, 0:256])
                    nc.vector.tensor_copy(v_sb[k + 1][:, 0:256], ps[:, 256:512])

            qp = {}

            def qproj(g, only_do=None):
                # wq packs at wvar indices 3,4,5; lhsT free dim = do chunk.
                # q-prep (hi / lo16 / hi16 quantization) runs straight from
                # PSUM on DVE: hi = e4m3(ps + bq), dq = ps - hi,
                # lo16 = e4m3(16*dq + bq*16... bias already in hi), so
                # dq must subtract hi from the biased value: dq = (ps+bq)-hi.
                if g in qp:
                    qph, qpl, qpu = qp[g]
                else:
                    qph = qpp.tile([128, 2, 512], e4, tag="qph", name=f"qph{g}")
                    qpl = qpp.tile([128, 2, 512], e4, tag="qpl", name=f"qpl{g}")
                    qpu = qpp.tile([128, 2, 512], e4, tag="qpu", name=f"qpu{g}")
                    qp[g] = (qph, qpl, qpu)
                dos = range(2) if only_do is None else [only_do]
                for do in dos:
                    ps = s_ps.tile([128, 512], dt, tag="sp", name=f"qp{g}{do}")
                    d0, d1 = do * 128, (do + 1) * 128
                    nc.tensor.matmul(ps, wvar(3)[:, :, d0:d1], avar(0, g),
                                     start=True, stop=False, perf_mode=DR)
                    nc.tensor.matmul(ps, wvar(5)[:, :, d0:d1], avar(1, g),
                                     start=False, stop=False, perf_mode=DR)
                    nc.tensor.matmul(ps, wvar(4)[:, :, d0:d1], avar(2, g),
                                     start=False, stop=True, perf_mode=DR)
                    hi = qph[:, do:do + 1, :]
                    bqs = bq_sb[:, do:do + 1]
                    dq = dqp.tile([128, 512], dt, tag="dq", name=f"dq{g}{do}")
                    if do == 0:
                        # hi = e4(ps+bq); dq = ps-hi; lo16 = e4((dq+bq)*16)
                        nc.vector.tensor_scalar_add(hi, ps, bqs)
                        nc.vector.tensor_sub(dq, ps, hi)
                        nc.vector.tensor_scalar(
                            qpl[:, do:do + 1, :], dq, bqs, 16.0,
                            mybir.AluOpType.add, mybir.AluOpType.mult)
                        nc.vector.tensor_scalar_mul(qpu[:, do:do + 1, :], hi,
                                                    0.0625)
                    else:
                        nc.scalar.activation(
                            out=hi, in_=ps,
                            func=mybir.ActivationFunctionType.Identity,
                            bias=bqs)
                        nc.vector.tensor_sub(dq, ps, hi)
                        nc.vector.tensor_scalar(
                            qpl[:, do:do + 1, :], dq, bqs, 16.0,
                            mybir.AluOpType.add, mybir.AluOpType.mult)
                        nc.scalar.mul(qpu[:, do:do + 1, :], hi, 0.0625)

            ets = [None] * KC

            def scores(g, k):
                qph, qpl, qpu = qp[g]
                sp = s_ps.tile([128, 512], dt, tag="sp", name=f"s{g}_{k}")
                c0, c1 = k * 128, (k + 1) * 128
                nc.tensor.matmul(sp, kvar(0, c0, c1), qph,
                                 start=True, stop=False, perf_mode=DR)
                nc.tensor.matmul(sp, kvar(2, c0, c1), qpl,
                                 start=False, stop=False, perf_mode=DR)
                nc.tensor.matmul(sp, kvar(1, c0, c1), qpu,
                                 start=False, stop=True, perf_mode=DR)
                et = expp.tile([128, 512], rdt, tag="et", name=f"e{g}_{k}")
                nc.scalar.activation(out=et, in_=sp, func=Exp)
                ets[k] = et

            o_acc = [o_ps.tile([128, 258], dt, tag=f"o{qs}", name=f"o{qs}")
                     for qs in range(4)]

            def av(g, k):
                order = range(4)
                if g == QG - 1 and k == KC - 1:
                    order = range(3, -1, -1)
                for qs in order:
                    nc.tensor.matmul(
                        o_acc[qs],
                        ets[k][:, qs * 128:(qs + 1) * 128],
                        v_sb[k][:, 0:258],
                        start=(k == 0), stop=(k == KC - 1),
                    )
                ets[k] = None

            def flush(g):
                last = g == QG - 1
                order = range(3, -1, -1) if last else range(4)
                for n, qs in enumerate(order):
                    ot = ostg.tile([128, 258], dt, tag=f"ot{qs}",
                                   name=f"ot{g}{qs}")
                    if last and n % 2 == 1:
                        nc.scalar.copy(ot, o_acc[qs])
                    else:
                        nc.vector.tensor_copy(ot, o_acc[qs])
                    # split store issue between HWDGE (SP) and SWDGE (Pool)
                    eng = nc.gpsimd if n == 1 else nc.sync
                    eng.dma_start(
                        out=out[(g * 4 + qs) * 128:(g * 4 + qs + 1) * 128, :],
                        in_=ot)

            # ---- schedule ----
            qproj(0)
            for k in range(0, 4, 2):
                vproj_pair(k)
            # group 0 (fused remaining vproj pairs), 2-deep scores pipeline
            next_v = [4]

            def maybe_vproj():
                if next_v[0] < KC:
                    vproj_pair(next_v[0])
                    next_v[0] += 2

            for g in range(QG):
                scores(g, 0)
                scores(g, 1)
                for k in range(KC):
                    if k >= 1:
                        av(g, k - 1)
                    if k + 2 < KC:
                        scores(g, k + 2)
                    if g == 0 and k % 2 == 1:
                        maybe_vproj()
                    if k == 14 and g < QG - 1:
                        qproj(g + 1, only_do=0)
                    if k == 18 and g < QG - 1:
                        qproj(g + 1, only_do=1)
                av(g, KC - 1)
                flush(g)
    nc.finalize()
    return nc


def _get_nc():
    if "nc" not in _COMPILED:
        _COMPILED["nc"] = _build_nc()
    return _COMPILED["nc"]


def _get_runner():
    """Jit the SPMD executable once and reuse it across kernel() calls."""
    if "runner" in _COMPILED:
        return _COMPILED["runner"]
    import jax
    from jax.experimental.shard_map import shard_map
    from jax.sharding import Mesh, PartitionSpec
    from concourse import bass2jax, mybir
    from concourse.bass2jax import _bass_exec_p, install_neuronx_cc_hook

    nc = _get_nc()
    install_neuronx_cc_hook()
    try:
        jax.config.update("jax_compilation_cache_dir", "/tmp/jax_cache")
        jax.config.update("jax_persistent_cache_min_compile_time_secs", 0.0)
        jax.config.update("jax_persistent_cache_min_entry_size_bytes", -1)
    except Exception:
        pass
    in_names, out_names, out_avals, zero_outs = [], [], [], []
    for alloc in nc.m.functions[0].allocations:
        if not isinstance(alloc, mybir.MemoryLocationSet):
            continue
        name = alloc.memorylocations[0].name
        if alloc.kind == "ExternalInput":
            if nc.partition_id_tensor is None or \
                    name != nc.partition_id_tensor.name:
                in_names.append(name)
        elif alloc.kind == "ExternalOutput":
            out_names.append(name)
            shape = tuple(alloc.tensor_shape)
            dtype = mybir.dt.np(alloc.dtype)
            out_avals.append(jax.core.ShapedArray(shape, dtype))
            zero_outs.append(np.zeros(shape, dtype))
    all_names = in_names + out_names
    if nc.partition_id_tensor is not None:
        all_names.append(nc.partition_id_tensor.name)

    def _body(*args):
        operands = list(args)
        if nc.partition_id_tensor is not None:
            operands.append(bass2jax.partition_id_tensor())
        return tuple(_bass_exec_p.bind(
            *operands, out_avals=tuple(out_avals), in_names=tuple(all_names),
            out_names=tuple(out_names), lowering_input_output_aliases=(),
            sim_require_finite=True, sim_require_nnan=True, nc=nc))

    devices = jax.devices()[:NCORES]
    mesh = Mesh(np.asarray(devices), ("core",))
    n_io = len(in_names) + len(out_names)
    sharded = jax.jit(
        shard_map(_body, mesh=mesh,
                  in_specs=(PartitionSpec("core"),) * n_io,
                  out_specs=(PartitionSpec("core"),) * len(out_names),
                  check_rep=False),
        keep_unused=True)
    _COMPILED["runner"] = (sharded, in_names, out_names, zero_outs)
    return _COMPILED["runner"]


def _split_pack(x):
    """x [128, 2, N] f32 -> concat([hi, lo16, hi16], axis=2) in e4m3."""
    import ml_dtypes
    E4 = ml_dtypes.float8_e4m3
    hi = x.astype(E4)
    hif = hi.astype(np.float32)
    lo16 = ((x - hif) * np.float32(16.0)).astype(E4)
    hi16 = (hif * np.float32(0.0625)).astype(E4)
    return np.concatenate([hi, lo16, hi16], axis=2)


def _chpack(x):
    """[C, N] -> [128, 2, N]: channel ch=i*128+p -> (p, i)."""
    return np.ascontiguousarray(
        x.reshape(2, 128, x.shape[1]).transpose(1, 0, 2))


def kernel(feat_A, feat_B, Wq, bq, Wk, bk, Wv, bv, Wo, bo, **_unused):
    f32 = np.float32
    fa = np.asarray(feat_A, f32).reshape(B, C, HW)
    fb = np.asarray(feat_B, f32).reshape(B, C, HW)
    # fold Wk into the Q projection and Wo into the V projection; the
    # (Q-bias . bk) cross term is a per-query constant, which softmax
    # ignores, so it is dropped exactly.
    Wq64 = np.asarray(Wq, np.float64) * SCALE
    Wk64 = np.asarray(Wk, np.float64)
    Wv64 = np.asarray(Wv, np.float64)
    Wo64 = np.asarray(Wo, np.float64)
    M = (Wq64.T @ Wk64) / SCALE                     # Wq^T Wk (unscaled)
    wv_t = (Wo64 @ Wv64).T                          # [C(in), C(d)]
    bq_s = (np.asarray(bq, np.float64) @ Wk64)
    bv_r = (Wo64 @ np.asarray(bv, np.float64)).astype(f32)
    bo_c = np.asarray(bo, f32)

    # device scaling: q_dev = (a*SA) @ (M*2) + bq*SQ  (sigma ~0.354)
    #                 k_dev = b*SQ;  v = k_dev^T @ (wv_t/SQ)
    SA = f32(SQ / 2)
    wq_dev = (M * 2.0).astype(f32)
    bq_dev = np.ascontiguousarray(
        (bq_s * SQ).astype(f32).reshape(2, 128).T)
    wv_dev = (wv_t / SQ).astype(f32)

    wpack = np.concatenate([
        _split_pack(_chpack(wv_dev)),
        _split_pack(_chpack(wq_dev)),
    ], axis=2)

    # k packs are shared by the two cores of each batch
    kpacks = [_split_pack(_chpack((fb[b] * f32(SQ)))) for b in range(B)]

    in_maps = []
    for cidx in range(NCORES):
        b, qh = cidx // 2, cidx % 2
        a_half = fa[b][:, qh * NQ:(qh + 1) * NQ] * SA
        in_maps.append({
            "kp": kpacks[b],
            "ap": _split_pack(_chpack(a_half)),
            "wp": wpack,
            "bq": bq_dev,
        })

    try:
        sharded, in_names, out_names, zero_outs = _get_runner()
        concat_in = [np.concatenate([in_maps[c][nm] for c in range(NCORES)],
                                    axis=0) for nm in in_names]
        concat_zeros = [np.zeros((NCORES * z.shape[0], *z.shape[1:]), z.dtype)
                        for z in zero_outs]
        out_arrs = sharded(*concat_in, *concat_zeros)
        res_out = np.asarray(out_arrs[out_names.index("out")]) \
            .reshape(NCORES, NQ, 258)
    except Exception:
        from concourse.bass_utils import run_bass_kernel_spmd
        res = run_bass_kernel_spmd(_get_nc(), in_maps, list(range(NCORES)))
        res_out = np.stack([res.results[c]["out"] for c in range(NCORES)])

    add_c = (bv_r + bo_c).astype(f32)               # [C]
    outf = np.empty((B, C, HW), f32)
    for cidx in range(NCORES):
        b, qh = cidx // 2, cidx % 2
        num = res_out[cidx][:, 0:256]
        den = res_out[cidx][:, 256:257]
        tok = num / den + add_c                     # [NQ, C]
        outf[b][:, qh * NQ:(qh + 1) * NQ] = tok.T
    return outf.reshape(B, C, 64, 64)


if __name__ == "__main__":
    rng = np.random.default_rng(0)
    ins = {
        "feat_A": rng.standard_normal((B, C, 64, 64), dtype=np.float32),
        "feat_B": rng.standard_normal((B, C, 64, 64), dtype=np.float32),
    }
    for nm in ("q", "k", "v", "o"):
        ins[f"W{nm}"] = rng.standard_normal((C, C), dtype=np.float32) / 16.0
        ins[f"b{nm}"] = np.zeros(C, np.float32)
    o = kernel(**ins)
    print("kernel ran, out shape", o.shape, "mean", float(np.abs(o).mean()))


# revision 14
# speedup vs baseline: 1.1857x; 1.0033x over previous
"""Cross-attention Trainium2 kernel (8 NeuronCores, SPMD).

Problem: B=4, C=256, H=W=64 -> N=4096 tokens/batch, single-head attention
over full C=256 with scale 1/sqrt(64)=1/8, then output projection.

Sharding: 2 cores per batch; each core owns 2048 queries (half the batch's
4096) and replicates K/V work for its batch (cheap vs. collectives).

v4 strategy: split-precision fp8 DoubleRow matmuls for scores and both
projections; fp32r for AV.

  - Wk folded into Q projection on host, Wo folded into Wv.
  - Every fp8 operand x ships as three e4m3 tensors: hi=fp8(x),
    lo16=fp8((x-hi)*16), hi16=fp8(hi/16).  A product x@y is computed as
    3 DoubleRow matmuls: hi@hi + hi16@lo16 + lo16@hi16 (exact power-of-2
    scale cancellation; dropped lo*lo term is ~1e-4 relative).
  - DoubleRow contracts 2x128=256 in ONE instruction at 0.5 cyc/row, so
    each 3-term product costs 75% of the fp32r pair while keeping
    ~4e-3 end-to-end relative error (measured in sim_fp8.py).
  - Scale balance: scores operands q,k at sigma~0.354 (sqrt(1/8) each),
    qproj operands a*0.177 / M*2, vproj k_dev / wv_t/0.354.
  - AV stays fp32r with the ones-column denominator trick; numerator +
    denominator ship unnormalized to DRAM; host does divide + bias +
    transpose.
  - V projection in chunk pairs fused into group 0's attention loop.
"""

import numpy as np

B, C, HW = 4, 256, 4096
NQ = HW // 2          # queries per core
NCORES = 8
KC = HW // 128        # 32 key chunks
QG = NQ // 512        # 4 query groups of 512 per core
SCALE = 1.0 / 8.0     # 1/sqrt(dim_head=64)
SQ = float(np.sqrt(SCALE))

_COMPILED = {}


def _build_nc():
    import concourse.bass as bass
    from concourse import bacc, mybir
    import concourse.tile as tile

    dt = mybir.dt.float32
    rdt = mybir.dt.float32r
    e4 = mybir.dt.float8e4
    DR = mybir.MatmulPerfMode.DoubleRow
    Exp = mybir.ActivationFunctionType.Exp

    nc = bacc.Bacc("TRN2", target_bir_lowering=False, debug=False)

    # k/a packs: [128, 2, 3*N]: dim1 = channel half (ch = i*128+p),
    # dim2 = variant-major: [0:N]=hi, [N:2N]=lo16, [2N:3N]=hi16
    kp = nc.dram_tensor("kp", [128, 2, 3 * HW], e4, kind="ExternalInput")
    ap_d = nc.dram_tensor("ap", [128, 2, 3 * NQ], e4, kind="ExternalInput")
    # weight packs: [0:256]=wvh, [256:512]=wvl16, [512:768]=wvh16,
    #               [768:1024]=wqh, [1024:1280]=wql16, [1280:1536]=wqh16
    wp = nc.dram_tensor("wp", [128, 2, 1536], e4, kind="ExternalInput")
    bqd = nc.dram_tensor("bq", [128, 2], dt, kind="ExternalInput")
    out = nc.dram_tensor("out", [NQ, 258], dt, kind="ExternalOutput")

    with tile.TileContext(nc) as tc:
        with (
            tc.tile_pool(name="consts", bufs=1) as consts,
            tc.tile_pool(name="feat", bufs=1) as feat,
            tc.tile_pool(name="qkt", bufs=1) as qkt,
            tc.tile_pool(name="qpp", bufs=2) as qpp,
            tc.tile_pool(name="dqp", bufs=2) as dqp,
            tc.tile_pool(name="vsb", bufs=1) as vsb,
            tc.tile_pool(name="expp", bufs=4) as expp,
            tc.tile_pool(name="ostg", bufs=1) as ostg,
            tc.tile_pool(name="vp_ps", bufs=1, space="PSUM") as vp_ps,
            tc.tile_pool(name="s_ps", bufs=3, space="PSUM") as s_ps,
            tc.tile_pool(name="o_ps", bufs=1, space="PSUM") as o_ps,
        ):
            kp_sb = feat.tile([128, 2, 3 * HW], e4, tag="kp", name="kp")
            ap_sb = feat.tile([128, 2, 3 * NQ], e4, tag="ap", name="ap")
            wp_sb = consts.tile([128, 2, 1536], e4, tag="wp", name="wp")
            bq_sb = consts.tile([128, 2], dt, tag="bq", name="bq")
            v_sb = [vsb.tile([128, 258], rdt, tag=f"v{k}", name=f"v{k}")
                    for k in range(KC)]

            def kvar(v, c0, c1):
                return kp_sb[:, :, v * HW + c0:v * HW + c1]

            def avar(v, g):
                return ap_sb[:, :, v * NQ + g * 512:v * NQ + (g + 1) * 512]

            def wvar(idx):
                return wp_sb[:, :, idx * 256:(idx + 1) * 256]

            # ---- DMA program (all on SP/HWDGE, priority order) ----
            def dma_k(c0, c1):
                for v in range(3):
                    nc.sync.dma_start(out=kvar(v, c0, c1),
                                      in_=kp[:, :, v * HW + c0:v * HW + c1])

            nc.sync.dma_start(out=wp_sb, in_=wp[:, :, :])
            nc.sync.dma_start(out=bq_sb, in_=bqd[:, :])
            for v in range(3):
                nc.sync.dma_start(
                    out=ap_sb[:, :, v * NQ:v * NQ + 512],
                    in_=ap_d[:, :, v * NQ:v * NQ + 512])
            dma_k(0, 512)
            dma_k(512, 1536)
            dma_k(1536, 2560)
            for v in range(3):
                nc.sync.dma_start(
                    out=ap_sb[:, :, v * NQ + 512:(v + 1) * NQ],
                    in_=ap_d[:, :, v * NQ + 512:(v + 1) * NQ])
            dma_k(2560, 4096)

            # ones columns for the AV denominator; ACT table warm-up
            ones = consts.tile([128, 2], dt, tag="ones")
            nc.gpsimd.memset(ones, 1.0)
            for k in range(KC):
                nc.gpsimd.tensor_copy(v_sb[k][:, 256:258], ones)
            warm = consts.tile([128, 1], dt, tag="warm")
            nc.scalar.activation(out=warm, in_=ones[:, 0:1], func=Exp)

            # ---- emission helpers ----
            vcnt = [0]

            def vproj_pair(k):
                # chunks k, k+1 into one [128,512] PSUM tile; one copy out
                ps = vp_ps.tile([128, 512], dt, tag="vp", name=f"vp{k}")
                for kk in (k, k + 1):
                    sl = ps[:, (kk - k) * 256:(kk - k + 1) * 256]
                    nc.tensor.matmul(sl, kvar(0, kk * 128, (kk + 1) * 128),
                                     wvar(0), start=True, stop=False,
                                     perf_mode=DR)
                    nc.tensor.matmul(sl, kvar(2, kk * 128, (kk + 1) * 128),
                                     wvar(1), start=False, stop=False,
                                     perf_mode=DR)
                    nc.tensor.matmul(sl, kvar(1, kk * 128, (kk + 1) * 128),
                                     wvar(2), start=False, stop=True,
                                     perf_mode=DR)
                nc.vector.tensor_copy(v_sb[k][:, 0:256], ps[:, 0:256])
                nc.vector.tensor_copy(v_sb[k + 1][:, 0:256], ps[:, 256:512])

            qp = {}

            def qproj(g, only_do=None):
                # wq packs at wvar indices 3,4,5; lhsT free dim = do chunk.
                # q-prep (hi / lo16 / hi16 quantization) runs straight from
                # PSUM on DVE: hi = e4m3(ps + bq), dq = ps - hi,
                # lo16 = e4m3(16*dq + bq*16... bias already in hi), so
                # dq must subtract hi from the biased value: dq = (ps+bq)-hi.
                if g in qp:
                    qph, qpl, qpu = qp[g]
                else:
                    qph = qpp.tile([128, 2, 512], e4, tag="qph", name=f"qph{g}")
                    qpl = qpp.tile([128, 2, 512], e4, tag="qpl", name=f"qpl{g}")
                    qpu = qpp.tile([128, 2, 512], e4, tag="qpu", name=f"qpu{g}")
                    qp[g] = (qph, qpl, qpu)
                dos = range(2) if only_do is None else [only_do]
                for do in dos:
                    ps = s_ps.tile([128, 512], dt, tag="sp", name=f"qp{g}{do}")
                    d0, d1 = do * 128, (do + 1) * 128
                    nc.tensor.matmul(ps, wvar(3)[:, :, d0:d1], avar(0, g),
                                     start=True, stop=False, perf_mode=DR)
                    nc.tensor.matmul(ps, wvar(5)[:, :, d0:d1], avar(1, g),
                                     start=False, stop=False, perf_mode=DR)
                    nc.tensor.matmul(ps, wvar(4)[:, :, d0:d1], avar(2, g),
                                     start=False, stop=True, perf_mode=DR)
                    hi = qph[:, do:do + 1, :]
                    bqs = bq_sb[:, do:do + 1]
                    dq = dqp.tile([128, 512], dt, tag="dq", name=f"dq{g}{do}")
                    if do == 0:
                        # hi = e4(ps+bq); dq = ps-hi; lo16 = e4((dq+bq)*16)
                        nc.vector.tensor_scalar_add(hi, ps, bqs)
                        nc.vector.tensor_sub(dq, ps, hi)
                        nc.vector.tensor_scalar(
                            qpl[:, do:do + 1, :], dq, bqs, 16.0,
                            mybir.AluOpType.add, mybir.AluOpType.mult)
                        nc.vector.tensor_scalar_mul(qpu[:, do:do + 1, :], hi,
                                                    0.0625)
                    else:
                        nc.scalar.activation(
                            out=hi, in_=ps,
                            func=mybir.ActivationFunctionType.Identity,
                            bias=bqs)
                        nc.vector.tensor_sub(dq, ps, hi)
                        nc.vector.tensor_scalar(
                            qpl[:, do:do + 1, :], dq, bqs, 16.0,
                            mybir.AluOpType.add, mybir.AluOpType.mult)
                        nc.scalar.mul(qpu[:, do:do + 1, :], hi, 0.0625)

            ets = [None] * KC

            def scores(g, k):
                qph, qpl, qpu = qp[g]
                sp = s_ps.tile([128, 512], dt, tag="sp", name=f"s{g}_{k}")
                c0, c1 = k * 128, (k + 1) * 128
                nc.tensor.matmul(sp, kvar(0, c0, c1), qph,
                                 start=True, stop=False, perf_mode=DR)
                nc.tensor.matmul(sp, kvar(2, c0, c1), qpl,
                                 start=False, stop=False, perf_mode=DR)
                nc.tensor.matmul(sp, kvar(1, c0, c1), qpu,
                                 start=False, stop=True, perf_mode=DR)
                et = expp.tile([128, 512], rdt, tag="et", name=f"e{g}_{k}")
                nc.scalar.activation(out=et, in_=sp, func=Exp)
                ets[k] = et

            o_acc = [o_ps.tile([128, 258], dt, tag=f"o{qs}", name=f"o{qs}")
                     for qs in range(4)]

            def av(g, k):
                order = range(4)
                if g == QG - 1 and k == KC - 1:
                    order = range(3, -1, -1)
                for qs in order:
                    nc.tensor.matmul(
                        o_acc[qs],
                        ets[k][:, qs * 128:(qs + 1) * 128],
                        v_sb[k][:, 0:258],
                        start=(k == 0), stop=(k == KC - 1),
                    )
                ets[k] = None

            def flush(g):
                last = g == QG - 1
                order = range(3, -1, -1) if last else range(4)
                for n, qs in enumerate(order):
                    ot = ostg.tile([128, 258], dt, tag=f"ot{qs}",
                                   name=f"ot{g}{qs}")
                    if last and n % 2 == 1:
                        nc.scalar.copy(ot, o_acc[qs])
                    else:
                        nc.vector.tensor_copy(ot, o_acc[qs])
                    # split store issue between HWDGE (SP) and SWDGE (Pool)
                    eng = nc.gpsimd if n == 1 else nc.sync
                    eng.dma_start(
                        out=out[(g * 4 + qs) * 128:(g * 4 + qs + 1) * 128, :],
                        in_=ot)

            # ---- schedule ----
            qproj(0)
            for k in range(0, 4, 2):
                vproj_pair(k)
            # group 0 (fused remaining vproj pairs), 2-deep scores pipeline
            next_v = [4]

            def maybe_vproj():
                if next_v[0] < KC:
                    vproj_pair(next_v[0])
                    next_v[0] += 2

            for g in range(QG):
                scores(g, 0)
                scores(g, 1)
                for k in range(KC):
                    if k >= 1:
                        av(g, k - 1)
                    if k + 2 < KC:
                        scores(g, k + 2)
                    if g == 0 and k % 2 == 1:
                        maybe_vproj()
                    if k == 14 and g < QG - 1:
                        qproj(g + 1, only_do=0)
                    if k == 18 and g < QG - 1:
                        qproj(g + 1, only_do=1)
                av(g, KC - 1)
                flush(g)
    nc.finalize()
    return nc


def _get_nc():
    if "nc" not in _COMPILED:
        _COMPILED["nc"] = _build_nc()
    return _COMPILED["nc"]


def _get_runner():
    """Jit the SPMD executable once and reuse it across kernel() calls."""
    if "runner" in _COMPILED:
        return _COMPILED["runner"]
    import jax
    from jax.experimental.shard_map import shard_map
    from jax.sharding import Mesh, PartitionSpec
    from concourse import bass2jax, mybir
    from concourse.bass2jax import _bass_exec_p, install_neuronx_cc_hook

    nc = _get_nc()
    install_neuronx_cc_hook()
    try:
        jax.config.update("jax_compilation_cache_dir", "/tmp/jax_cache")
        jax.config.update("jax_persistent_cache_min_compile_time_secs", 0.0)
        jax.config.update("jax_persistent_cache_min_entry_size_bytes", -1)
    except Exception:
        pass
    in_names, out_names, out_avals, zero_outs = [], [], [], []
    for alloc in nc.m.functions[0].allocations:
        if not isinstance(alloc, mybir.MemoryLocationSet):
            continue
        name = alloc.memorylocations[0].name
        if alloc.kind == "ExternalInput":
            if nc.partition_id_tensor is None or \
                    name != nc.partition_id_tensor.name:
                in_names.append(name)
        elif alloc.kind == "ExternalOutput":
            out_names.append(name)
            shape = tuple(alloc.tensor_shape)
            dtype = mybir.dt.np(alloc.dtype)
            out_avals.append(jax.core.ShapedArray(shape, dtype))
            zero_outs.append(np.zeros(shape, dtype))
    all_names = in_names + out_names
    if nc.partition_id_tensor is not None:
        all_names.append(nc.partition_id_tensor.name)

    def _body(*args):
        operands = list(args)
        if nc.partition_id_tensor is not None:
            operands.append(bass2jax.partition_id_tensor())
        return tuple(_bass_exec_p.bind(
            *operands, out_avals=tuple(out_avals), in_names=tuple(all_names),
            out_names=tuple(out_names), lowering_input_output_aliases=(),
            sim_require_finite=True, sim_require_nnan=True, nc=nc))

    devices = jax.devices()[:NCORES]
    mesh = Mesh(np.asarray(devices), ("core",))
    n_io = len(in_names) + len(out_names)
    sharded = jax.jit(
        shard_map(_body, mesh=mesh,
                  in_specs=(PartitionSpec("core"),) * n_io,
                  out_specs=(PartitionSpec("core"),) * len(out_names),
                  check_rep=False),
        keep_unused=True)
    _COMPILED["runner"] = (sharded, in_names, out_names, zero_outs)
    return _COMPILED["runner"]


def _split_pack(x):
    """x [128, 2, N] f32 -> concat([hi, lo16, hi16], axis=2) in e4m3."""
    import ml_dtypes
    E4 = ml_dtypes.float8_e4m3
    hi = x.astype(E4)
    hif = hi.astype(np.float32)
    lo16 = ((x - hif) * np.float32(16.0)).astype(E4)
    hi16 = (hif * np.float32(0.0625)).astype(E4)
    return np.concatenate([hi, lo16, hi16], axis=2)


def _chpack(x):
    """[C, N] -> [128, 2, N]: channel ch=i*128+p -> (p, i)."""
    return np.ascontiguousarray(
        x.reshape(2, 128, x.shape[1]).transpose(1, 0, 2))


def kernel(feat_A, feat_B, Wq, bq, Wk, bk, Wv, bv, Wo, bo, **_unused):
    f32 = np.float32
    fa = np.asarray(feat_A, f32).reshape(B, C, HW)
    fb = np.asarray(feat_B, f32).reshape(B, C, HW)
    # fold Wk into the Q projection and Wo into the V projection; the
    # (Q-bias . bk) cross term is a per-query constant, which softmax
    # ignores, so it is dropped exactly.
    Wq64 = np.asarray(Wq, np.float64) * SCALE
    Wk64 = np.asarray(Wk, np.float64)
    Wv64 = np.asarray(Wv, np.float64)
    Wo64 = np.asarray(Wo, np.float64)
    M = (Wq64.T @ Wk64) / SCALE                     # Wq^T Wk (unscaled)
    wv_t = (Wo64 @ Wv64).T                          # [C(in), C(d)]
    bq_s = (np.asarray(bq, np.float64) @ Wk64)
    bv_r = (Wo64 @ np.asarray(bv, np.float64)).astype(f32)
    bo_c = np.asarray(bo, f32)

    # device scaling: q_dev = (a*SA) @ (M*2) + bq*SQ  (sigma ~0.354)
    #                 k_dev = b*SQ;  v = k_dev^T @ (wv_t/SQ)
    SA = f32(SQ / 2)
    wq_dev = (M * 2.0).astype(f32)
    bq_dev = np.ascontiguousarray(
        (bq_s * SQ).astype(f32).reshape(2, 128).T)
    wv_dev = (wv_t / SQ).astype(f32)

    wpack = np.concatenate([
        _split_pack(_chpack(wv_dev)),
        _split_pack(_chpack(wq_dev)),
    ], axis=2)

    # k packs are shared by the two cores of each batch
    kpacks = [_split_pack(_chpack((fb[b] * f32(SQ)))) for b in range(B)]

    in_maps = []
    for cidx in range(NCORES):
        b, qh = cidx // 2, cidx % 2
        a_half = fa[b][:, qh * NQ:(qh + 1) * NQ] * SA
        in_maps.append({
            "kp": kpacks[b],
            "ap": _split_pack(_chpack(a_half)),
            "wp": wpack,
            "bq": bq_dev,
        })

    try:
        sharded, in_names, out_names, zero_outs = _get_runner()
        concat_in = [np.concatenate([in_maps[c][nm] for c in range(NCORES)],
                                    axis=0) for nm in in_names]
        concat_zeros = [np.zeros((NCORES * z.shape[0], *z.shape[1:]), z.dtype)
                        for z in zero_outs]
        out_arrs = sharded(*concat_in, *concat_zeros)
        res_out = np.asarray(out_arrs[out_names.index("out")]) \
            .reshape(NCORES, NQ, 258)
    except Exception:
        from concourse.bass_utils import run_bass_kernel_spmd
        res = run_bass_kernel_spmd(_get_nc(), in_maps, list(range(NCORES)))
        res_out = np.stack([res.results[c]["out"] for c in range(NCORES)])

    add_c = (bv_r + bo_c).astype(f32)               # [C]
    outf = np.empty((B, C, HW), f32)
    for cidx in range(NCORES):
        b, qh = cidx // 2, cidx % 2
        num = res_out[cidx][:, 0:256]
        den = res_out[cidx][:, 256:257]
        tok = num / den + add_c                     # [NQ, C]
        outf[b][:, qh * NQ:(qh + 1) * NQ] = tok.T
    return outf.reshape(B, C, 64, 64)


if __name__ == "__main__":
    rng = np.random.default_rng(0)
    ins = {
        "feat_A": rng.standard_normal((B, C, 64, 64), dtype=np.float32),
        "feat_B": rng.standard_normal((B, C, 64, 64), dtype=np.float32),
    }
    for nm in ("q", "k", "v", "o"):
        ins[f"W{nm}"] = rng.standard_normal((C, C), dtype=np.float32) / 16.0
        ins[f"b{nm}"] = np.zeros(C, np.float32)
    o = kernel(**ins)
    print("kernel ran, out shape", o.shape, "mean", float(np.abs(o).mean()))
